# Initial kernel scaffold
#
"""Trainium2 Bass kernel for nn_AnyNetRefinement (disparity refinement with SPN scan).

Data-parallel over batch: core b processes image b end-to-end (no collectives).
Pipeline per core:
  conv1..conv3 (3x3+BN+ReLU, bf16, row-stacked PE matmuls)
  conv4 -> raw gates G -> normalize (|G1|+|G2|+|G3|) -> A (tap-major, bf16)
  convd (disp -> 8ch feature, f32)
  w0 = (1-a-b-c)*dfeat (f32)
  SPN left-to-right scan over W=640 on VectorE (folded [128=(c,hblock), 26] state,
    3-tap multiply via multi-dim AP + tensor_reduce + stream_shuffle halos)
  convc (prop -> residual) + disp + relu -> out
"""

import numpy as np
import ml_dtypes

BF = ml_dtypes.bfloat16

H, W = 384, 640
HP, WP = 387, 642        # padded activation planes (+1 top/left, +2 bottom, +1 right)
X0S = (0, 320)           # x-half starts
NX = 320                 # matmul free size (psum-bank safe)

_CACHE = {}


# ---------------------------------------------------------------- host helpers
def _fold_bn(wt, g, b, m, v):
    s = g / np.sqrt(v + 1e-5)
    return (wt * s.reshape(-1, 1, 1, 1)).astype(np.float32), (b - m * s).astype(np.float32)


def _lhsT_dxk(wt, r_out, r_in):
    """lhsT [K=(cin,dx,yi), M=(cout,r_out)] for dx-in-K conv."""
    cout, cin = wt.shape[0], wt.shape[1]
    K = cin * 3 * r_in
    M = cout * r_out
    out = np.zeros((K, M), np.float32)
    for c in range(cin):
        for dx in range(3):
            for yi in range(r_in):
                k = (c * 3 + dx) * r_in + yi
                for co in range(cout):
                    for yo in range(r_out):
                        dy = yi - yo
                        if 0 <= dy <= 2:
                            out[k, co * r_out + yo] = wt[co, c, dy, dx]
    return out


def _lhsT_dx3(wt, r_out, r_in):
    """3 lhsT mats [K=(cin,yi), M=(cout,r_out)], one per dx; returned [K, 3, M]."""
    cout, cin = wt.shape[0], wt.shape[1]
    K = cin * r_in
    M = cout * r_out
    out = np.zeros((K, 3, M), np.float32)
    for dx in range(3):
        for c in range(cin):
            for yi in range(r_in):
                k = c * r_in + yi
                for co in range(cout):
                    for yo in range(r_out):
                        dy = yi - yo
                        if 0 <= dy <= 2:
                            out[k, dx, co * r_out + yo] = wt[co, c, dy, dx]
    return out


def _pad_img(x, hp=HP, wp=WP):
    # x [C,H,W] -> [C,hp,wp] with +1 offset (zero border)
    out = np.zeros((x.shape[0], hp, wp), BF)
    out[:, 1:1 + H, 1:1 + W] = x.astype(BF)
    return out


# ---------------------------------------------------------------- bass builder
def _build():
    import concourse.bass as bass
    import concourse.mybir as mybir
    from concourse import tile
    from concourse.vector_clock import ScopedClock

    f32 = mybir.dt.float32
    bf16 = mybir.dt.bfloat16
    ALU = mybir.AluOpType
    ACTF = mybir.ActivationFunctionType
    AX = mybir.AxisListType

    class TC(tile.TileContext):
        # this walrus build accepts only one sync-wait on the final Drain;
        # split the end-of-kernel waits across several drains.
        def _drain_and_barrier(self, tick_clock, wait_clock):
            nc = self.nc
            drain_inst = nc.sync.drain()
            wait_clock.add_sem_waits(drain_inst.ins, ScopedClock({None: tick_clock.global_clock}))
            waits = list(drain_inst.ins.sync_info.on_wait)
            if len(waits) > 1:
                drain_inst.ins.sync_info.on_wait = waits[:1]
                for i in range(1, len(waits)):
                    d2 = nc.sync.drain()
                    if d2.ins.sync_info is None:
                        d2.ins.sync_info = mybir.SyncInfo(on_wait=[waits[i]], on_update=[])
                    else:
                        d2.ins.sync_info.on_wait = [waits[i]]
            nc.all_engine_barrier()
            popped = nc._tile_sem_poison_stack.pop()
            assert popped is self._sem_poison
            nc.clear_and_free_semaphores(list(self.sems.allocated().values()))
            nc.all_engine_barrier()

    def mkap(t, offset, dims):
        # t: AP (base of a dram tile / tensor); dims: [(stride, count), ...]
        return bass.AP(t.tensor, t.offset + offset, [list(d) for d in dims])

    nc = bass.Bass("TRN2")

    # ---------------- dram parameters
    img = nc.declare_dram_parameter("img", [3, HP, WP], bf16, isOutput=False)
    dpad = nc.declare_dram_parameter("dpad", [1, HP, WP], bf16, isOutput=False)
    dispf = nc.declare_dram_parameter("dispf", [H, W], f32, isOutput=False)
    w1k = nc.declare_dram_parameter("w1k", [90, 128], bf16, isOutput=False)
    w2k = nc.declare_dram_parameter("w2k", [128, 3 * 96], bf16, isOutput=False)
    w3k = nc.declare_dram_parameter("w3k", [128, 3 * 96], bf16, isOutput=False)
    w4k = nc.declare_dram_parameter("w4k", [112, 3 * 120], bf16, isOutput=False)
    wdk = nc.declare_dram_parameter("wdk", [54, 128], bf16, isOutput=False)
    wck = nc.declare_dram_parameter("wck", [108, 2 * 16], bf16, isOutput=False)
    b1v = nc.declare_dram_parameter("b1v", [128, 1], f32, isOutput=False)
    b2v = nc.declare_dram_parameter("b2v", [96, 1], f32, isOutput=False)
    b3v = nc.declare_dram_parameter("b3v", [96, 1], f32, isOutput=False)
    outp = nc.declare_dram_parameter("out", [H, W], f32, isOutput=True)

    with TC(nc) as tc:
        dram = tc.tile_pool(name="dram", bufs=1, space="DRAM").__enter__()
        act1 = dram.tile([16, HP, WP], bf16, tag="act1")
        act2 = dram.tile([16, HP, WP], bf16, tag="act2")
        act3 = dram.tile([16, HP, WP], bf16, tag="act3")
        Gt = dram.tile([24, 385, W], bf16, tag="G")
        dfeat = dram.tile([8, H, W], f32, tag="dfeat")
        Amat = dram.tile([3, 8, H, W], bf16, tag="Amat")
        W0t = dram.tile([8, H, W], f32, tag="W0")
        ppad = dram.tile([8, HP, WP], bf16, tag="ppad")

        pw = tc.tile_pool(name="wts", bufs=1).__enter__()
        prhs = tc.tile_pool(name="rhs", bufs=3).__enter__()
        pout = tc.tile_pool(name="cout", bufs=3).__enter__()
        ppsum = tc.tile_pool(name="psum", bufs=8, space="PSUM").__enter__()
        pnorm = tc.tile_pool(name="norm", bufs=2).__enter__()
        pscan = tc.tile_pool(name="scan", bufs=2).__enter__()
        pst = tc.tile_pool(name="state", bufs=1).__enter__()

        # ---------------- load weights/biases
        w1t = pw.tile([90, 128], bf16, tag="w1t")
        nc.sync.dma_start(out=w1t[:], in_=w1k[:])
        w2t = pw.tile([128, 3, 96], bf16, tag="w2t")
        nc.sync.dma_start(out=w2t[:], in_=w2k[:].rearrange("k (d m) -> k d m", d=3))
        w3t = pw.tile([128, 3, 96], bf16, tag="w3t")
        nc.sync.dma_start(out=w3t[:], in_=w3k[:].rearrange("k (d m) -> k d m", d=3))
        w4t = pw.tile([112, 3, 120], bf16, tag="w4t")
        nc.sync.dma_start(out=w4t[:], in_=w4k[:].rearrange("k (d m) -> k d m", d=3))
        wdt = pw.tile([54, 128], bf16, tag="wdt")
        nc.sync.dma_start(out=wdt[:], in_=wdk[:])
        wct = pw.tile([108, 2, 16], bf16, tag="wct")
        nc.sync.dma_start(out=wct[:], in_=wck[:].rearrange("k (d m) -> k d m", d=2))
        b1t = pw.tile([128, 1], f32, tag="b1t")
        nc.sync.dma_start(out=b1t[:], in_=b1v[:])
        b2t = pw.tile([96, 1], f32, tag="b2t")
        nc.sync.dma_start(out=b2t[:], in_=b2v[:])
        b3t = pw.tile([96, 1], f32, tag="b3t")
        nc.sync.dma_start(out=b3t[:], in_=b3v[:])

        # ---------------- zero borders of padded internal buffers
        zt = pw.tile([128, WP], bf16, tag="zt")
        nc.vector.memset(zt[:], 0.0)
        for buf, cc in ((act1, 16), (act2, 16), (act3, 16), (ppad, 8)):
            # rows 0, 385, 386
            nc.sync.dma_start(out=mkap(buf, 0, [(HP * WP, cc), (1, WP)]),
                              in_=mkap(zt, 0, [(WP, cc), (1, WP)]))
            nc.sync.dma_start(out=mkap(buf, 385 * WP, [(HP * WP, cc), (1, 2 * WP)]),
                              in_=mkap(zt, 0, [(0, cc), (0, 2), (1, WP)]).opt())
            # cols 0 and 641
            nc.sync.dma_start(out=mkap(buf, 0, [(HP * WP, cc), (WP, HP)]),
                              in_=mkap(zt, 0, [(WP, cc), (0, HP)]))
            nc.sync.dma_start(out=mkap(buf, WP - 1, [(HP * WP, cc), (WP, HP)]),
                              in_=mkap(zt, 0, [(WP, cc), (0, HP)]))

        # ---------------- conv layers
        def conv_dx3(src, dst, wtile, btile, cin, cout, r, rin, S, GS, relu, dst_plane, dst_w, dst_off):
            K = cin * rin
            M = cout * r
            ep = [0]
            g0 = 0
            while g0 < S:
                nsl = min(GS, S - g0)
                y0 = r * g0
                rhs = prhs.tile([K, GS, WP], bf16, tag="rhs")
                nc.sync.dma_start(
                    out=rhs[:, :nsl, :],
                    in_=mkap(src, y0 * WP,
                             [(HP * WP, cin), (WP, rin), (r * WP, nsl), (1, WP)]))
                ps = []
                for xh in range(2):
                    for dx in range(3):
                        for sl in range(nsl):
                            if dx == 0:
                                ps.append(ppsum.tile([128, NX], f32, tag="ps"))
                            nc.tensor.matmul(
                                ps[xh * nsl + sl][:M, :],
                                wtile[:, dx, :],
                                rhs[:, sl, X0S[xh] + dx:X0S[xh] + dx + NX],
                                start=(dx == 0), stop=(dx == 2))
                ot = pout.tile([M, GS, 2 * NX], bf16, tag="cout")
                for xh in range(2):
                    for sl in range(nsl):
                        p = ps[xh * nsl + sl][:M, :]
                        o = ot[:, sl, xh * NX:(xh + 1) * NX]
                        if relu:
                            if ep[0] % 2 == 0:
                                nc.vector.tensor_scalar(o, p, btile[:M, :], 0.0, ALU.add, ALU.max)
                            else:
                                nc.scalar.activation(o, p, ACTF.Relu, bias=btile[:M, :], scale=1.0)
                        else:
                            if ep[0] % 2 == 0:
                                nc.vector.tensor_copy(o, p)
                            else:
                                nc.scalar.copy(o, p)
                        ep[0] += 1
                nc.sync.dma_start(
                    out=mkap(dst, dst_off + y0 * dst_w,
                             [(dst_plane, cout), (dst_w, r), (r * dst_w, nsl), (1, NX)]),
                    in_=ot[:, :nsl, 0:NX])
                nc.sync.dma_start(
                    out=mkap(dst, dst_off + y0 * dst_w + NX,
                             [(dst_plane, cout), (dst_w, r), (r * dst_w, nsl), (1, NX)]),
                    in_=ot[:, :nsl, NX:2 * NX])
                g0 += nsl

        def conv_dxk(src, dst, wtiles, cin_g, npass, cout, r, rin, S, GS, src_cstride,
                     dst_plane, dst_w, dst_off, out_f32=False, disp_add=False):
            # wtiles: list of lhsT APs (len npass); K = cin_g*3*rin per pass
            K = cin_g * 3 * rin
            M = cout * r
            ep = [0]
            g0 = 0
            while g0 < S:
                nsl = min(GS, S - g0)
                y0 = r * g0
                rhss = []
                for p in range(npass):
                    rt = prhs.tile([K, GS, W], bf16, tag="rhsk")
                    nc.sync.dma_start(
                        out=rt[:, :nsl, :],
                        in_=mkap(src, p * cin_g * src_cstride + y0 * WP,
                                 [(src_cstride, cin_g), (1, 3), (WP, rin), (r * WP, nsl), (1, W)]))
                    rhss.append(rt)
                ps = []
                for xh in range(2):
                    for p in range(npass):
                        for sl in range(nsl):
                            if p == 0:
                                ps.append(ppsum.tile([128, NX], f32, tag="ps"))
                            nc.tensor.matmul(
                                ps[xh * nsl + sl][:M, :],
                                wtiles[p],
                                rhss[p][:, sl, X0S[xh]:X0S[xh] + NX],
                                start=(p == 0), stop=(p == npass - 1))
                ot = pout.tile([M, GS, 2 * NX], f32 if out_f32 else bf16, tag="coutk")
                for xh in range(2):
                    for sl in range(nsl):
                        p = ps[xh * nsl + sl][:M, :]
                        o = ot[:, sl, xh * NX:(xh + 1) * NX]
                        if disp_add:
                            dt_ = pnorm.tile([16, NX], f32, tag="dtile")
                            nc.sync.dma_start(
                                out=dt_[:],
                                in_=mkap(dispf, (y0 + sl * r) * W + X0S[xh], [(W, 16), (1, NX)]))
                            tmp = pnorm.tile([16, NX], f32, tag="ctmp")
                            nc.vector.tensor_tensor(out=tmp[:], in0=p, in1=dt_[:], op=ALU.add)
                            nc.vector.tensor_scalar(o, tmp[:], 0.0, None, ALU.max)
                        elif ep[0] % 2 == 0:
                            nc.vector.tensor_copy(o, p)
                        else:
                            nc.scalar.copy(o, p)
                        ep[0] += 1
                for xh in range(2):
                    nc.sync.dma_start(
                        out=mkap(dst, dst_off + y0 * dst_w + xh * NX,
                                 [(dst_plane, cout), (dst_w, r), (r * dst_w, nsl), (1, NX)]),
                        in_=ot[:, :nsl, xh * NX:(xh + 1) * NX])
                g0 += nsl

        # conv1: img -> act1
        conv_dxk(img, act1, [w1t[:]], 3, 1, 16, 8, 10, 48, 4, HP * WP,
                 HP * WP, WP, WP + 1)
        tc.strict_bb_all_engine_barrier()
        # conv2: act1 -> act2
        conv_dx3(act1, act2, w2t, b2t, 16, 16, 6, 8, 64, 4, True, HP * WP, WP, WP + 1)
        tc.strict_bb_all_engine_barrier()
        # conv3: act2 -> act3
        conv_dx3(act2, act3, w3t, b3t, 16, 16, 6, 8, 64, 4, True, HP * WP, WP, WP + 1)
        tc.strict_bb_all_engine_barrier()
        # conv4: act3 -> G (no bias/relu); G rows 0..384 (row 384 junk)
        conv_dx3(act3, Gt, w4t, None, 16, 24, 5, 7, 77, 4, False, 385 * W, W, 0)
        # convd: dpad -> dfeat (f32)
        conv_dxk(dpad, dfeat, [wdt[:]], 1, 1, 8, 16, 18, 24, 4, HP * WP,
                 H * W, W, 0, out_f32=True)
        tc.strict_bb_all_engine_barrier()

        # ---------------- gate normalization -> Amat, W0t
        NCH = 4
        CH = 15360 // NCH  # 3840 per-partition elems per chunk
        GP = 385 * W       # G channel plane
        for k in range(NCH):
            gts = []
            for tap in range(3):
                g = pnorm.tile([128, CH], bf16, tag="gld")
                nc.sync.dma_start(
                    out=g[:],
                    in_=mkap(Gt, tap * 8 * GP + k * CH, [(GP, 8), (24 * W, 16), (1, CH)]))
                gts.append(g)
            ab = []
            for tap in range(3):
                a = pnorm.tile([128, CH], bf16, tag="gabs")
                nc.scalar.activation(a[:], gts[tap][:], ACTF.Abs)
                ab.append(a)
            s12 = pnorm.tile([128, CH], bf16, tag="s12")
            nc.vector.tensor_tensor(out=s12[:], in0=ab[0][:], in1=ab[1][:], op=ALU.add)
            sf = pnorm.tile([128, CH], f32, tag="sf")
            nc.vector.scalar_tensor_tensor(out=sf[:], in0=ab[2][:], scalar=1e-8,
                                           in1=s12[:], op0=ALU.add, op1=ALU.add)
            rs = pnorm.tile([128, CH], f32, tag="rs")
            nc.vector.reciprocal_approx_fast(out=rs[:], in_=sf[:])
            An = []
            for tap in range(3):
                a = pnorm.tile([128, CH], bf16, tag="An")
                nc.vector.tensor_tensor(out=a[:], in0=gts[tap][:], in1=rs[:], op=ALU.mult)
                nc.sync.dma_start(
                    out=mkap(Amat, tap * 8 * H * W + k * CH, [(H * W, 8), (24 * W, 16), (1, CH)]),
                    in_=a[:])
                An.append(a)
            a12 = pnorm.tile([128, CH], bf16, tag="a12")
            nc.vector.tensor_tensor(out=a12[:], in0=An[0][:], in1=An[1][:], op=ALU.add)
            asum = pnorm.tile([128, CH], bf16, tag="asum")
            nc.vector.tensor_tensor(out=asum[:], in0=a12[:], in1=An[2][:], op=ALU.add)
            t2 = pnorm.tile([128, CH], bf16, tag="t2")
            nc.vector.tensor_scalar(t2[:], asum[:], -1.0, 1.0, ALU.mult, ALU.add)
            df = pnorm.tile([128, CH], f32, tag="dfl")
            nc.sync.dma_start(out=df[:],
                              in_=mkap(dfeat, k * CH, [(H * W, 8), (24 * W, 16), (1, CH)]))
            w0c = pnorm.tile([128, CH], f32, tag="w0c")
            nc.vector.tensor_tensor(out=w0c[:], in0=t2[:], in1=df[:], op=ALU.mult)
            nc.sync.dma_start(out=mkap(W0t, k * CH, [(H * W, 8), (24 * W, 16), (1, CH)]),
                              in_=w0c[:])
        # zero edge gates: A1 row 0, A3 row 383
        nc.sync.dma_start(out=mkap(Amat, 0, [(H * W, 8), (1, W)]),
                          in_=mkap(zt, 0, [(WP, 8), (1, W)]))
        nc.sync.dma_start(out=mkap(Amat, 2 * 8 * H * W + 383 * W, [(H * W, 8), (1, W)]),
                          in_=mkap(zt, 0, [(WP, 8), (1, W)]))
        tc.strict_bb_all_engine_barrier()

        # ---------------- SPN scan
        TS = 160
        state = pst.tile([128, 2, 26], f32, tag="st")
        nc.vector.memset(state[:], 0.0)
        mask_up = [(i - 1) % 32 for i in range(32)]
        mask_dn = [(i + 1) % 32 for i in range(32)]
        for ck in range(W // TS):
            t0 = ck * TS
            gwin = pscan.tile([128, TS, 24, 3], bf16, tag="gwin")
            nc.sync.dma_start(
                out=gwin[:],
                in_=mkap(Amat, t0, [(H * W, 8), (24 * W, 16), (1, TS), (W, 24), (8 * H * W, 3)]))
            wwin = pscan.tile([128, TS, 24], f32, tag="wwin")
            nc.sync.dma_start(
                out=wwin[:],
                in_=mkap(W0t, t0, [(H * W, 8), (24 * W, 16), (1, TS), (W, 24)]))
            pbuf = pscan.tile([128, TS, 24], bf16, tag="pbuf")
            for j in range(TS):
                t = t0 + j
                cur, nxt = t % 2, (t + 1) % 2
                prod = pscan.tile([128, 24, 3], f32, tag="prod")
                taps = bass.AP(state.tensor, state.offset + cur * 26, [[1, 128], [1, 24], [1, 3]])
                nc.vector.tensor_tensor(out=prod[:], in0=gwin[:, j], in1=taps, op=ALU.mult)
                acc = pscan.tile([128, 24], f32, tag="acc")
                nc.vector.tensor_reduce(out=acc[:], in_=prod[:], axis=AX.X, op=ALU.add)
                nc.vector.tensor_tensor(out=state[:, nxt, 1:25], in0=acc[:], in1=wwin[:, j], op=ALU.add)
                nc.vector.stream_shuffle(out=state[:, nxt, 0:1], in_=state[:, nxt, 24:25], mask=mask_up)
                nc.vector.stream_shuffle(out=state[:, nxt, 25:26], in_=state[:, nxt, 1:2], mask=mask_dn)
                nc.scalar.copy(pbuf[:, j, :], state[:, nxt, 1:25])
            nc.sync.dma_start(
                out=mkap(ppad, WP + 1 + t0, [(HP * WP, 8), (24 * WP, 16), (1, TS), (WP, 24)]),
                in_=pbuf[:])
        tc.strict_bb_all_engine_barrier()

        # ---------------- convc: ppad -> out (+disp, relu)
        conv_dxk(ppad, outp, [wct[:, 0, :], wct[:, 1, :]], 4, 2, 1, 16, 18, 24, 4,
                 HP * WP, H * W, W, 0, out_f32=True, disp_add=True)

    return nc


def _prep_inputs(inputs):
    """host-side: returns list of 8 in_maps."""
    w1, b1 = _fold_bn(inputs['w1'], inputs['bn1_g'], inputs['bn1_b'], inputs['bn1_m'], inputs['bn1_v'])
    w2, b2 = _fold_bn(inputs['w2'], inputs['bn2_g'], inputs['bn2_b'], inputs['bn2_m'], inputs['bn2_v'])
    w3, b3 = _fold_bn(inputs['w3'], inputs['bn3_g'], inputs['bn3_b'], inputs['bn3_m'], inputs['bn3_v'])

    w1k = _lhsT_dxk(w1, 8, 10).astype(BF)                      # [90,128]
    w2k = _lhsT_dx3(w2, 6, 8).reshape(128, 3 * 96).astype(BF)
    w3k = _lhsT_dx3(w3, 6, 8).reshape(128, 3 * 96).astype(BF)
    w4k = _lhsT_dx3(inputs['w4'].astype(np.float32), 5, 7).reshape(112, 3 * 120).astype(BF)
    wdk = _lhsT_dxk(inputs['wd'].astype(np.float32), 16, 18).astype(BF)  # [54,128]
    # convc: 2 passes of 4 input channels
    wc = inputs['wc'].astype(np.float32)
    wck = np.zeros((108, 2, 16), np.float32)
    for p in range(2):
        wck[:, p, :] = _lhsT_dxk(wc[:, 4 * p:4 * p + 4], 16, 18)
    wck = wck.reshape(108, 32).astype(BF)

    b1r = np.repeat(b1, 8).reshape(128, 1).astype(np.float32)
    b2r = np.repeat(b2, 6).reshape(96, 1).astype(np.float32)
    b3r = np.repeat(b3, 6).reshape(96, 1).astype(np.float32)

    maps = []
    for b in range(8):
        maps.append({
            "img": _pad_img(inputs['leftImage'][b]),
            "dpad": _pad_img(inputs['disp'][b]),
            "dispf": inputs['disp'][b, 0].astype(np.float32),
            "w1k": w1k, "w2k": w2k, "w3k": w3k, "w4k": w4k, "wdk": wdk, "wck": wck,
            "b1v": b1r, "b2v": b2r, "b3v": b3r,
        })
    return maps


def kernel(**inputs):
    from concourse.bass_utils import run_bass_kernel_spmd

    if "nc" not in _CACHE:
        _CACHE["nc"] = _build()
    nc = _CACHE["nc"]
    maps = _prep_inputs(inputs)
    res = run_bass_kernel_spmd(nc, maps, core_ids=list(range(8)))
    out = np.stack([res.results[i]["out"] for i in range(8)])[:, None].astype(np.float32)
    return out


# revision 7
# speedup vs baseline: 1.0047x; 1.0047x over previous
"""Trainium2 Bass kernel for nn_AnyNetRefinement (disparity refinement with SPN scan).

Data-parallel over batch: core b processes image b end-to-end (no collectives).
Pipeline per core:
  conv1..conv3 (3x3+BN+ReLU, bf16, row-stacked PE matmuls)
  conv4 -> raw gates G -> normalize (|G1|+|G2|+|G3|) -> A (tap-major, bf16)
  convd (disp -> 8ch feature, f32)
  w0 = (1-a-b-c)*dfeat (f32)
  SPN left-to-right scan over W=640 on VectorE (folded [128=(c,hblock), 26] state,
    3-tap multiply via multi-dim AP + tensor_reduce + stream_shuffle halos)
  convc (prop -> residual) + disp + relu -> out
"""

import numpy as np
import ml_dtypes

BF = ml_dtypes.bfloat16

H, W = 384, 640
HP, WP = 387, 642        # padded activation planes (+1 top/left, +2 bottom, +1 right)
X0S = (0, 320)
NX = 320                 # matmul free size (psum-bank safe)

_CACHE = {}


# ---------------------------------------------------------------- host helpers
def _fold_bn(wt, g, b, m, v):
    s = g / np.sqrt(v + 1e-5)
    return (wt * s.reshape(-1, 1, 1, 1)).astype(np.float32), (b - m * s).astype(np.float32)


def _lhsT(wt, r_out, r_in, cin_g, npass):
    """lhsT [K=(cin_g,yi), npass, 3, M=(cout,r_out)]."""
    cout, cin = wt.shape[0], wt.shape[1]
    K = cin_g * r_in
    M = cout * r_out
    out = np.zeros((K, npass, 3, M), np.float32)
    for p in range(npass):
        for cg in range(cin_g):
            c = p * cin_g + cg
            if c >= cin:
                continue
            for dx in range(3):
                for yi in range(r_in):
                    k = cg * r_in + yi
                    for co in range(cout):
                        for yo in range(r_out):
                            dy = yi - yo
                            if 0 <= dy <= 2:
                                out[k, p, dx, co * r_out + yo] = wt[co, c, dy, dx]
    return out.reshape(K, npass * 3 * M)


def _pad_img(x, hp=HP, wp=WP):
    out = np.zeros((x.shape[0], hp, wp), BF)
    out[:, 1:1 + H, 1:1 + W] = x.astype(BF)
    return out


# ---------------------------------------------------------------- bass builder
def _build():
    import concourse.bass as bass
    import concourse.mybir as mybir
    from concourse import tile
    from concourse.vector_clock import ScopedClock

    f32 = mybir.dt.float32
    bf16 = mybir.dt.bfloat16
    ALU = mybir.AluOpType
    ACTF = mybir.ActivationFunctionType
    AX = mybir.AxisListType

    class TC(tile.TileContext):
        # this walrus build accepts only one sync-wait per Drain; split the
        # end-of-kernel waits across several drains.
        def _drain_and_barrier(self, tick_clock, wait_clock):
            nc = self.nc
            drain_inst = nc.sync.drain()
            wait_clock.add_sem_waits(drain_inst.ins, ScopedClock({None: tick_clock.global_clock}))
            waits = list(drain_inst.ins.sync_info.on_wait)
            if len(waits) > 1:
                drain_inst.ins.sync_info.on_wait = waits[:1]
                for i in range(1, len(waits)):
                    d2 = nc.sync.drain()
                    if d2.ins.sync_info is None:
                        d2.ins.sync_info = mybir.SyncInfo(on_wait=[waits[i]], on_update=[])
                    else:
                        d2.ins.sync_info.on_wait = [waits[i]]
            nc.all_engine_barrier()
            popped = nc._tile_sem_poison_stack.pop()
            assert popped is self._sem_poison
            nc.clear_and_free_semaphores(list(self.sems.allocated().values()))
            nc.all_engine_barrier()

    def dap(t, offset, dims):
        base = t if isinstance(t, bass.AP) else t[:]
        return bass.AP(base.tensor, base.offset + offset, [list(d) for d in dims])

    def sap(tile_ap, nparts, offset, dims):
        pstep = tile_ap.ap[0][0]
        return bass.AP(tile_ap.tensor, tile_ap.offset + offset,
                       [[pstep, nparts]] + [list(d) for d in dims])

    nc = bass.Bass("TRN2")

    img = nc.declare_dram_parameter("img", [3, HP, WP], bf16, isOutput=False)
    dpad = nc.declare_dram_parameter("dpad", [1, HP, WP], bf16, isOutput=False)
    dispf = nc.declare_dram_parameter("dispf", [H, W], f32, isOutput=False)
    w1k = nc.declare_dram_parameter("w1k", [30, 3 * 128], bf16, isOutput=False)
    w2k = nc.declare_dram_parameter("w2k", [128, 3 * 96], bf16, isOutput=False)
    w3k = nc.declare_dram_parameter("w3k", [128, 3 * 96], bf16, isOutput=False)
    w4k = nc.declare_dram_parameter("w4k", [112, 3 * 120], bf16, isOutput=False)
    wdk = nc.declare_dram_parameter("wdk", [18, 3 * 128], bf16, isOutput=False)
    wck = nc.declare_dram_parameter("wck", [72, 2 * 3 * 16], bf16, isOutput=False)
    b1v = nc.declare_dram_parameter("b1v", [128, 1], f32, isOutput=False)
    b2v = nc.declare_dram_parameter("b2v", [96, 1], f32, isOutput=False)
    b3v = nc.declare_dram_parameter("b3v", [96, 1], f32, isOutput=False)
    outp = nc.declare_dram_parameter("out", [H, W], f32, isOutput=True)

    with TC(nc) as tc:
        with (tc.tile_pool(name="dram", bufs=1, space="DRAM") as dram,
              tc.tile_pool(name="wts", bufs=1) as pw):
            act1 = dram.tile([16, HP, WP], bf16, tag="act1")
            act2 = dram.tile([16, HP, WP], bf16, tag="act2")
            act3 = dram.tile([16, HP, WP], bf16, tag="act3")
            Gt = dram.tile([24, 385, W], bf16, tag="G")
            dfeat = dram.tile([8, H, W], f32, tag="dfeat")
            Amat = dram.tile([3, 8, H, W], bf16, tag="Amat")
            W0t = dram.tile([8, H, W], f32, tag="W0")
            ppad = dram.tile([8, HP, WP], bf16, tag="ppad")

            # ---------------- weights/biases
            wtl = {}
            for nm, prm, kk, nm3 in (("w1", w1k, 30, 3 * 128), ("w2", w2k, 128, 3 * 96),
                                     ("w3", w3k, 128, 3 * 96), ("w4", w4k, 112, 3 * 120),
                                     ("wd", wdk, 18, 3 * 128), ("wc", wck, 72, 6 * 16)):
                t = pw.tile([kk, nm3], bf16, tag=f"{nm}t", name=f"{nm}t")
                nc.sync.dma_start(out=t[:], in_=prm[:])
                wtl[nm] = t
            b1t = pw.tile([128, 1], f32, tag="b1t")
            nc.sync.dma_start(out=b1t[:], in_=b1v[:])
            b2t = pw.tile([96, 1], f32, tag="b2t")
            nc.sync.dma_start(out=b2t[:], in_=b2v[:])
            b3t = pw.tile([96, 1], f32, tag="b3t")
            nc.sync.dma_start(out=b3t[:], in_=b3v[:])

            # ---------------- zero row-borders of padded internal buffers
            zt = pw.tile([128, 2 * WP], bf16, tag="zt")
            nc.vector.memset(zt[:], 0.0)
            ztv = zt[:]
            for buf, cc in ((act1, 16), (act2, 16), (act3, 16), (ppad, 8)):
                nc.sync.dma_start(out=dap(buf, 0, [(HP * WP, cc), (1, WP)]),
                                  in_=sap(ztv, cc, 0, [(1, WP)]))
                nc.sync.dma_start(out=dap(buf, 385 * WP, [(HP * WP, cc), (1, 2 * WP)]),
                                  in_=sap(ztv, cc, 0, [(1, 2 * WP)]))

            if True:
                # ---------------- generic conv
                def conv(prhs, pout, ppsum, src, dst, wt, wK, wM, btile, cin_g, npass,
                         cout, r, rin, S, GS,
                         relu, dst_plane, dst_w, dst_pad, out_f32=False, disp_add=False):
                    K = cin_g * rin
                    assert K == wK
                    M = cout * r
                    assert M == wM
                    wv = wt[:]
                    ow = WP if dst_pad else W
                    g0 = 0
                    while g0 < S:
                        nsl = min(GS, S - g0)
                        y0 = r * g0
                        rhss = []
                        for p_ in range(npass):
                            rt = prhs.tile([K, GS, WP], bf16, tag="rhs", name="rhs")
                            for sl in range(nsl):
                                nc.sync.dma_start(
                                    out=rt[:, sl, :],
                                    in_=dap(src, p_ * cin_g * HP * WP + (y0 + sl * r) * WP,
                                            [(HP * WP, cin_g), (WP, rin), (1, WP)]))
                            rhss.append(rt)
                        ps = []
                        for xh in range(2):
                            for sl in range(nsl):
                                pstile = ppsum.tile([128, NX], f32, tag="ps", name="ps")
                                ps.append(pstile)
                        for xh in range(2):
                            for p_ in range(npass):
                                for dx in range(3):
                                    for sl in range(nsl):
                                        nc.tensor.matmul(
                                            ps[xh * nsl + sl][:M, :],
                                            sap(wv, K, (p_ * 3 + dx) * M, [(1, M)]),
                                            rhss[p_][:, sl, X0S[xh] + dx:X0S[xh] + dx + NX],
                                            start=(p_ == 0 and dx == 0),
                                            stop=(p_ == npass - 1 and dx == 2))
                        ot = pout.tile([M, GS, ow], f32 if out_f32 else bf16, tag="cout", name="cout")
                        if dst_pad:
                            nc.vector.memset(ot[:, :, 0:1], 0.0)
                            nc.vector.memset(ot[:, :, ow - 1:ow], 0.0)
                        for xh in range(2):
                            for sl in range(nsl):
                                p = ps[xh * nsl + sl][:M, :]
                                xb = (1 if dst_pad else 0) + xh * NX
                                o = ot[:, sl, xb:xb + NX]
                                if disp_add:
                                    dt_ = pout.tile([16, NX], f32, tag="dtile", name="dtile")
                                    nc.sync.dma_start(
                                        out=dt_[:],
                                        in_=dap(dispf, (y0 + sl * r) * W + X0S[xh], [(W, 16), (1, NX)]))
                                    tmp = pout.tile([16, NX], f32, tag="ctmp", name="ctmp")
                                    nc.vector.tensor_tensor(out=tmp[:], in0=p, in1=dt_[:], op=ALU.add)
                                    nc.vector.tensor_scalar(o, tmp[:], 0.0, None, ALU.max)
                                elif relu:
                                    nc.vector.tensor_scalar(o, p, btile[:M, :], 0.0, ALU.add, ALU.max)
                                else:
                                    nc.vector.tensor_copy(o, p)
                        for sl in range(nsl):
                            nc.scalar.dma_start(
                                out=dap(dst, ((1 if dst_pad else 0) + y0 + sl * r) * dst_w,
                                        [(dst_plane, cout), (dst_w, r), (1, ow)]),
                                in_=ot[:, sl, :])
                        g0 += nsl

                with (tc.tile_pool(name="rhs", bufs=3) as prhs,
                      tc.tile_pool(name="cout", bufs=3) as pout,
                      tc.tile_pool(name="psum", bufs=8, space="PSUM") as ppsum):
                    P3 = (prhs, pout, ppsum)
                    conv(*P3, img, act1, wtl["w1"], 30, 128, b1t, 3, 1, 16, 8, 10, 48, 4,
                         True, HP * WP, WP, True)
                    tc.strict_bb_all_engine_barrier()
                    conv(*P3, act1, act2, wtl["w2"], 128, 96, b2t, 16, 1, 16, 6, 8, 64, 4,
                         True, HP * WP, WP, True)
                    tc.strict_bb_all_engine_barrier()
                    conv(*P3, act2, act3, wtl["w3"], 128, 96, b3t, 16, 1, 16, 6, 8, 64, 4,
                         True, HP * WP, WP, True)
                    tc.strict_bb_all_engine_barrier()
                    conv(*P3, act3, Gt, wtl["w4"], 112, 120, None, 16, 1, 24, 5, 7, 77, 4,
                         False, 385 * W, W, False)
                    conv(*P3, dpad, dfeat, wtl["wd"], 18, 128, None, 1, 1, 8, 16, 18, 24, 4,
                         False, H * W, W, False, out_f32=True)
                    tc.strict_bb_all_engine_barrier()

                # ---------------- gate normalization -> Amat, W0t
                NCH = 8
                CH = 15360 // NCH
                GP = 385 * W
                with (tc.tile_pool(name="norm3", bufs=4) as pn3,
                      tc.tile_pool(name="norm1", bufs=1) as pn1):
                    for k in range(NCH):
                        gts = []
                        for tap in range(3):
                            g = pn3.tile([128, CH], bf16, tag="gld", name="gld")
                            nc.sync.dma_start(
                                out=g[:],
                                in_=dap(Gt, tap * 8 * GP + k * CH,
                                        [(GP, 8), (24 * W, 16), (1, CH)]))
                            gts.append(g)
                        ab = []
                        for tap in range(3):
                            a = pn3.tile([128, CH], bf16, tag="gabs", name="gabs")
                            nc.scalar.activation(a[:], gts[tap][:], ACTF.Abs)
                            ab.append(a)
                        s12 = pn1.tile([128, CH], bf16, tag="s12")
                        nc.vector.tensor_tensor(out=s12[:], in0=ab[0][:], in1=ab[1][:], op=ALU.add)
                        sf = pn1.tile([128, CH], f32, tag="sf")
                        nc.vector.scalar_tensor_tensor(out=sf[:], in0=ab[2][:], scalar=1e-8,
                                                       in1=s12[:], op0=ALU.add, op1=ALU.add)
                        rs = pn1.tile([128, CH], f32, tag="rs")
                        nc.vector.reciprocal_approx_fast(out=rs[:], in_=sf[:])
                        An = []
                        for tap in range(3):
                            a = pn3.tile([128, CH], bf16, tag="An", name="An")
                            nc.vector.tensor_tensor(out=a[:], in0=gts[tap][:], in1=rs[:], op=ALU.mult)
                            nc.scalar.dma_start(
                                out=dap(Amat, tap * 8 * H * W + k * CH,
                                        [(H * W, 8), (24 * W, 16), (1, CH)]),
                                in_=a[:])
                            An.append(a)
                        a12 = pn1.tile([128, CH], bf16, tag="a12")
                        nc.vector.tensor_tensor(out=a12[:], in0=An[0][:], in1=An[1][:], op=ALU.add)
                        asum = pn1.tile([128, CH], bf16, tag="asum")
                        nc.vector.tensor_tensor(out=asum[:], in0=a12[:], in1=An[2][:], op=ALU.add)
                        t2 = pn1.tile([128, CH], bf16, tag="t2")
                        nc.vector.tensor_scalar(t2[:], asum[:], -1.0, 1.0, ALU.mult, ALU.add)
                        df = pn1.tile([128, CH], f32, tag="dfl")
                        nc.sync.dma_start(out=df[:],
                                          in_=dap(dfeat, k * CH, [(H * W, 8), (24 * W, 16), (1, CH)]))
                        w0c = pn1.tile([128, CH], f32, tag="w0c")
                        nc.vector.tensor_tensor(out=w0c[:], in0=t2[:], in1=df[:], op=ALU.mult)
                        nc.scalar.dma_start(out=dap(W0t, k * CH, [(H * W, 8), (24 * W, 16), (1, CH)]),
                                            in_=w0c[:])
                    # zero edge gates: A1 row 0, A3 row 383
                    nc.sync.dma_start(out=dap(Amat, 0, [(H * W, 8), (1, W)]),
                                      in_=sap(ztv, 8, 0, [(1, W)]))
                    nc.sync.dma_start(out=dap(Amat, 2 * 8 * H * W + 383 * W, [(H * W, 8), (1, W)]),
                                      in_=sap(ztv, 8, 0, [(1, W)]))
                tc.strict_bb_all_engine_barrier()

                # ---------------- SPN scan (fully SBUF-resident windows)
                mask_up = [(i - 1) % 32 for i in range(32)]
                mask_dn = [(i + 1) % 32 for i in range(32)]
                with (tc.tile_pool(name="scanbig", bufs=1) as pbig,
                      tc.tile_pool(name="scansm", bufs=2) as psm):
                    gw = pbig.tile([128, 3, 24, W], bf16, tag="gw")
                    gwv = gw[:]
                    for tap in range(3):
                        nc.sync.dma_start(
                            out=gw[:, tap],
                            in_=dap(Amat, tap * 8 * H * W, [(H * W, 8), (24 * W, 16), (1, 24 * W)]))
                    w0w = pbig.tile([128, 24, W], f32, tag="w0w")
                    w0v = w0w[:]
                    nc.sync.dma_start(
                        out=w0w[:],
                        in_=dap(W0t, 0, [(H * W, 8), (24 * W, 16), (1, 24 * W)]))
                    pf = pbig.tile([128, 24, WP], bf16, tag="pf")
                    pfv = pf[:]
                    nc.vector.memset(pf[:, :, 0:1], 0.0)
                    nc.vector.memset(pf[:, :, WP - 1:WP], 0.0)
                    state = pbig.tile([128, 2, 26], f32, tag="st")
                    stv = state[:]
                    nc.vector.memset(stv, 0.0)
                    for t in range(W):
                        cur, nxt = t % 2, (t + 1) % 2
                        prod = psm.tile([128, 24, 3], f32, tag="prod")
                        taps = sap(stv, 128, cur * 26, [(1, 24), (1, 3)])
                        g_t = sap(gwv, 128, t, [(W, 24), (24 * W, 3)])
                        nc.vector.tensor_tensor(out=prod[:], in0=g_t, in1=taps, op=ALU.mult)
                        acc = psm.tile([128, 24], f32, tag="acc")
                        nc.vector.tensor_reduce(out=acc[:], in_=prod[:], axis=AX.X, op=ALU.add)
                        w_t = sap(w0v, 128, t, [(W, 24)])
                        nc.vector.tensor_tensor(out=state[:, nxt, 1:25], in0=acc[:],
                                                in1=w_t, op=ALU.add)
                        nc.vector.stream_shuffle(out=state[:, nxt, 0:1],
                                                 in_=state[:, nxt, 24:25], mask=mask_up)
                        nc.vector.stream_shuffle(out=state[:, nxt, 25:26],
                                                 in_=state[:, nxt, 1:2], mask=mask_dn)
                        p_t = sap(pfv, 128, 1 + t, [(WP, 24)])
                        nc.scalar.copy(p_t, state[:, nxt, 1:25])
                    # export prop -> ppad rows 1..384, full width
                    nc.sync.dma_start(
                        out=dap(ppad, WP, [(HP * WP, 8), (24 * WP, 16), (1, 24 * WP)]),
                        in_=pf[:])
                tc.strict_bb_all_engine_barrier()

                # ---------------- convc: ppad -> out (+disp, relu)
                with (tc.tile_pool(name="rhsc", bufs=3) as prhs2,
                      tc.tile_pool(name="coutc", bufs=3) as pout2,
                      tc.tile_pool(name="psumc", bufs=8, space="PSUM") as ppsum2):
                    conv(prhs2, pout2, ppsum2, ppad, outp, wtl["wc"], 72, 16, None, 4, 2,
                         1, 16, 18, 24, 4,
                         False, H * W, W, False, out_f32=True, disp_add=True)

    return nc


def _prep_inputs(inputs):
    w1, b1 = _fold_bn(inputs['w1'], inputs['bn1_g'], inputs['bn1_b'], inputs['bn1_m'], inputs['bn1_v'])
    w2, b2 = _fold_bn(inputs['w2'], inputs['bn2_g'], inputs['bn2_b'], inputs['bn2_m'], inputs['bn2_v'])
    w3, b3 = _fold_bn(inputs['w3'], inputs['bn3_g'], inputs['bn3_b'], inputs['bn3_m'], inputs['bn3_v'])

    w1k = _lhsT(w1, 8, 10, 3, 1).astype(BF)                       # [30, 384]
    w2k = _lhsT(w2, 6, 8, 16, 1).astype(BF)                       # [128, 288]
    w3k = _lhsT(w3, 6, 8, 16, 1).astype(BF)
    w4k = _lhsT(inputs['w4'].astype(np.float32), 5, 7, 16, 1).astype(BF)   # [112, 360]
    wdk = _lhsT(inputs['wd'].astype(np.float32), 16, 18, 1, 1).astype(BF)  # [18, 384]
    wck = _lhsT(inputs['wc'].astype(np.float32), 16, 18, 4, 2).astype(BF)  # [72, 96]

    b1r = np.repeat(b1, 8).reshape(128, 1).astype(np.float32)
    b2r = np.repeat(b2, 6).reshape(96, 1).astype(np.float32)
    b3r = np.repeat(b3, 6).reshape(96, 1).astype(np.float32)

    maps = []
    for b in range(8):
        maps.append({
            "img": _pad_img(inputs['leftImage'][b]),
            "dpad": _pad_img(inputs['disp'][b]),
            "dispf": inputs['disp'][b, 0].astype(np.float32),
            "w1k": w1k, "w2k": w2k, "w3k": w3k, "w4k": w4k, "wdk": wdk, "wck": wck,
            "b1v": b1r, "b2v": b2r, "b3v": b3r,
        })
    return maps


def kernel(**inputs):
    from concourse.bass_utils import run_bass_kernel_spmd

    if "nc" not in _CACHE:
        _CACHE["nc"] = _build()
    nc = _CACHE["nc"]
    maps = _prep_inputs(inputs)
    res = run_bass_kernel_spmd(nc, maps, core_ids=list(range(8)))
    out = np.stack([res.results[i]["out"] for i in range(8)])[:, None].astype(np.float32)
    return out


# revision 11
# speedup vs baseline: 1.0465x; 1.0416x over previous
"""Trainium2 Bass kernel for nn_AnyNetRefinement (disparity refinement with SPN scan).

Data-parallel over batch: core b processes image b end-to-end (no collectives).
Pipeline per core:
  conv1..conv3 (3x3+BN+ReLU, bf16, row-stacked PE matmuls, DRAM-padded acts)
  conv4 -> raw gates G; convd (disp -> 8ch feature, f32)
  normalize gates (|G1|+|G2|+|G3|) writing A taps + w0 directly into
    scan-resident SBUF tiles
  SPN left-to-right scan over W=640 on VectorE (folded [128=(c,hblock), 26] state,
    3-tap multiply into a slot buffer + 4-slot tensor_reduce (slot 3 = w0,
    pre-staged by ScalarE) + stream_shuffle halos)
  convc (prop -> residual) + disp + relu -> out
"""

import numpy as np
import ml_dtypes

BF = ml_dtypes.bfloat16

H, W = 384, 640
HP, WP = 387, 642        # padded activation planes (+1 top/left, +2 bottom, +1 right)
X0S = (0, 320)
NX = 320                 # matmul free size (psum-bank safe)

_CACHE = {}


# ---------------------------------------------------------------- host helpers
def _fold_bn(wt, g, b, m, v):
    s = g / np.sqrt(v + 1e-5)
    return (wt * s.reshape(-1, 1, 1, 1)).astype(np.float32), (b - m * s).astype(np.float32)


def _lhsT(wt, r_out, r_in, cin_g, npass):
    """lhsT [K=(cin_g,yi), npass, 3, M=(cout,r_out)]."""
    cout, cin = wt.shape[0], wt.shape[1]
    K = cin_g * r_in
    M = cout * r_out
    out = np.zeros((K, npass, 3, M), np.float32)
    for p in range(npass):
        for cg in range(cin_g):
            c = p * cin_g + cg
            if c >= cin:
                continue
            for dx in range(3):
                for yi in range(r_in):
                    k = cg * r_in + yi
                    for co in range(cout):
                        for yo in range(r_out):
                            dy = yi - yo
                            if 0 <= dy <= 2:
                                out[k, p, dx, co * r_out + yo] = wt[co, c, dy, dx]
    return out.reshape(K, npass * 3 * M)


def _pad_img(x, hp=HP, wp=WP):
    out = np.zeros((x.shape[0], hp, wp), BF)
    out[:, 1:1 + H, 1:1 + W] = x.astype(BF)
    return out


# ---------------------------------------------------------------- bass builder
def _build():
    import concourse.bass as bass
    import concourse.mybir as mybir
    from concourse import tile
    from concourse.vector_clock import ScopedClock

    f32 = mybir.dt.float32
    bf16 = mybir.dt.bfloat16
    ALU = mybir.AluOpType
    ACTF = mybir.ActivationFunctionType
    AX = mybir.AxisListType

    class TC(tile.TileContext):
        # this walrus build accepts only one sync-wait per Drain; split the
        # end-of-kernel waits across several drains.
        def _drain_and_barrier(self, tick_clock, wait_clock):
            nc = self.nc
            drain_inst = nc.sync.drain()
            wait_clock.add_sem_waits(drain_inst.ins, ScopedClock({None: tick_clock.global_clock}))
            waits = list(drain_inst.ins.sync_info.on_wait)
            if len(waits) > 1:
                drain_inst.ins.sync_info.on_wait = waits[:1]
                for i in range(1, len(waits)):
                    d2 = nc.sync.drain()
                    if d2.ins.sync_info is None:
                        d2.ins.sync_info = mybir.SyncInfo(on_wait=[waits[i]], on_update=[])
                    else:
                        d2.ins.sync_info.on_wait = [waits[i]]
            nc.all_engine_barrier()
            popped = nc._tile_sem_poison_stack.pop()
            assert popped is self._sem_poison
            nc.clear_and_free_semaphores(list(self.sems.allocated().values()))
            nc.all_engine_barrier()

    def dap(t, offset, dims):
        base = t if isinstance(t, bass.AP) else t[:]
        return bass.AP(base.tensor, base.offset + offset, [list(d) for d in dims])

    def sap(tile_ap, nparts, offset, dims, pstride=1):
        pstep = tile_ap.ap[0][0]
        return bass.AP(tile_ap.tensor, tile_ap.offset + offset,
                       [[pstep * pstride, nparts]] + [list(d) for d in dims])

    nc = bass.Bass("TRN2")

    img = nc.declare_dram_parameter("img", [3, HP, WP], bf16, isOutput=False)
    dpad = nc.declare_dram_parameter("dpad", [1, HP, WP], bf16, isOutput=False)
    dispf = nc.declare_dram_parameter("dispf", [H, W], f32, isOutput=False)
    w1k = nc.declare_dram_parameter("w1k", [30, 3 * 128], bf16, isOutput=False)
    w2k = nc.declare_dram_parameter("w2k", [128, 3 * 96], bf16, isOutput=False)
    w3k = nc.declare_dram_parameter("w3k", [128, 3 * 96], bf16, isOutput=False)
    w4k = nc.declare_dram_parameter("w4k", [112, 3 * 120], bf16, isOutput=False)
    wdk = nc.declare_dram_parameter("wdk", [18, 3 * 128], bf16, isOutput=False)
    wck = nc.declare_dram_parameter("wck", [72, 2 * 3 * 16], bf16, isOutput=False)
    b1v = nc.declare_dram_parameter("b1v", [128, 1], f32, isOutput=False)
    b2v = nc.declare_dram_parameter("b2v", [96, 1], f32, isOutput=False)
    b3v = nc.declare_dram_parameter("b3v", [96, 1], f32, isOutput=False)
    outp = nc.declare_dram_parameter("out", [H, W], f32, isOutput=True)

    with TC(nc) as tc:
        with (tc.tile_pool(name="dram", bufs=1, space="DRAM") as dram,
              tc.tile_pool(name="wts", bufs=1) as pw):
            act1 = dram.tile([16, HP, WP], bf16, tag="act1")
            act2 = dram.tile([16, HP, WP], bf16, tag="act2")
            act3 = dram.tile([16, HP, WP], bf16, tag="act3")
            Gt = dram.tile([24, 385, W], bf16, tag="G")
            dfeat = dram.tile([8, H, W], f32, tag="dfeat")
            ppad = dram.tile([8, HP, WP], bf16, tag="ppad")

            # ---------------- weights/biases
            wtl = {}
            for nm, prm, kk, nm3 in (("w1", w1k, 30, 3 * 128), ("w2", w2k, 128, 3 * 96),
                                     ("w3", w3k, 128, 3 * 96), ("w4", w4k, 112, 3 * 120),
                                     ("wd", wdk, 18, 3 * 128), ("wc", wck, 72, 6 * 16)):
                t = pw.tile([kk, nm3], bf16, tag=f"{nm}t", name=f"{nm}t")
                nc.sync.dma_start(out=t[:], in_=prm[:])
                wtl[nm] = t
            b1t = pw.tile([128, 1], f32, tag="b1t")
            nc.sync.dma_start(out=b1t[:], in_=b1v[:])
            b2t = pw.tile([96, 1], f32, tag="b2t")
            nc.sync.dma_start(out=b2t[:], in_=b2v[:])
            b3t = pw.tile([96, 1], f32, tag="b3t")
            nc.sync.dma_start(out=b3t[:], in_=b3v[:])

            # ---------------- zero row-borders of padded internal buffers
            zt = pw.tile([128, 2 * WP], bf16, tag="zt")
            nc.vector.memset(zt[:], 0.0)
            ztv = zt[:]
            for buf, cc in ((act1, 16), (act2, 16), (act3, 16), (ppad, 8)):
                nc.sync.dma_start(out=dap(buf, 0, [(HP * WP, cc), (1, WP)]),
                                  in_=sap(ztv, cc, 0, [(1, WP)]))
                nc.sync.dma_start(out=dap(buf, 385 * WP, [(HP * WP, cc), (1, 2 * WP)]),
                                  in_=sap(ztv, cc, 0, [(1, 2 * WP)]))

            # ---------------- generic conv
            rhs_eng = [0]

            def conv(prhs, pout, ppsum, src, dst, wt, wK, wM, btile, cin_g, npass,
                     cout, r, rin, S, GS,
                     relu, dst_plane, dst_w, dst_pad, out_f32=False, disp_add=False):
                K = cin_g * rin
                assert K == wK
                M = cout * r
                assert M == wM
                wv = wt[:]
                ow = WP if dst_pad else W
                g0 = 0
                while g0 < S:
                    nsl = min(GS, S - g0)
                    y0 = r * g0
                    rhss = []
                    for p_ in range(npass):
                        rt = prhs.tile([K, GS, WP], bf16, tag="rhs", name="rhs")
                        for sl in range(nsl):
                            eng = nc.sync if (rhs_eng[0] % 2 == 0) else nc.gpsimd
                            rhs_eng[0] += 1
                            eng.dma_start(
                                out=rt[:, sl, :],
                                in_=dap(src, p_ * cin_g * HP * WP + (y0 + sl * r) * WP,
                                        [(HP * WP, cin_g), (WP, rin), (1, WP)]))
                        rhss.append(rt)
                    ps = []
                    for xh in range(2):
                        for sl in range(nsl):
                            pstile = ppsum.tile([128, NX], f32, tag="ps", name="ps")
                            ps.append(pstile)
                    for xh in range(2):
                        for p_ in range(npass):
                            for dx in range(3):
                                for sl in range(nsl):
                                    nc.tensor.matmul(
                                        ps[xh * nsl + sl][:M, :],
                                        sap(wv, K, (p_ * 3 + dx) * M, [(1, M)]),
                                        rhss[p_][:, sl, X0S[xh] + dx:X0S[xh] + dx + NX],
                                        start=(p_ == 0 and dx == 0),
                                        stop=(p_ == npass - 1 and dx == 2))
                    ot = pout.tile([M, GS, ow], f32 if out_f32 else bf16, tag="cout", name="cout")
                    if dst_pad:
                        nc.vector.memset(ot[:, :, 0:1], 0.0)
                        nc.vector.memset(ot[:, :, ow - 1:ow], 0.0)
                    for xh in range(2):
                        for sl in range(nsl):
                            p = ps[xh * nsl + sl][:M, :]
                            xb = (1 if dst_pad else 0) + xh * NX
                            o = ot[:, sl, xb:xb + NX]
                            if disp_add:
                                dt_ = pout.tile([16, NX], f32, tag="dtile", name="dtile")
                                nc.gpsimd.dma_start(
                                    out=dt_[:],
                                    in_=dap(dispf, (y0 + sl * r) * W + X0S[xh], [(W, 16), (1, NX)]))
                                tmp = pout.tile([16, NX], f32, tag="ctmp", name="ctmp")
                                nc.vector.tensor_tensor(out=tmp[:], in0=p, in1=dt_[:], op=ALU.add)
                                nc.vector.tensor_scalar(o, tmp[:], 0.0, None, ALU.max)
                            elif relu:
                                nc.vector.tensor_scalar(o, p, btile[:M, :], 0.0, ALU.add, ALU.max)
                            else:
                                nc.vector.tensor_copy(o, p)
                    for sl in range(nsl):
                        nc.scalar.dma_start(
                            out=dap(dst, ((1 if dst_pad else 0) + y0 + sl * r) * dst_w,
                                    [(dst_plane, cout), (dst_w, r), (1, ow)]),
                            in_=ot[:, sl, :])
                    g0 += nsl

            with (tc.tile_pool(name="rhs", bufs=3) as prhs,
                  tc.tile_pool(name="cout", bufs=3) as pout,
                  tc.tile_pool(name="psum", bufs=8, space="PSUM") as ppsum):
                P3 = (prhs, pout, ppsum)
                conv(*P3, img, act1, wtl["w1"], 30, 128, b1t, 3, 1, 16, 8, 10, 48, 4,
                     True, HP * WP, WP, True)
                tc.strict_bb_all_engine_barrier()
                conv(*P3, act1, act2, wtl["w2"], 128, 96, b2t, 16, 1, 16, 6, 8, 64, 4,
                     True, HP * WP, WP, True)
                tc.strict_bb_all_engine_barrier()
                conv(*P3, act2, act3, wtl["w3"], 128, 96, b3t, 16, 1, 16, 6, 8, 64, 4,
                     True, HP * WP, WP, True)
                tc.strict_bb_all_engine_barrier()
                conv(*P3, act3, Gt, wtl["w4"], 112, 120, None, 16, 1, 24, 5, 7, 77, 4,
                     False, 385 * W, W, False)
                conv(*P3, dpad, dfeat, wtl["wd"], 18, 128, None, 1, 1, 8, 16, 18, 24, 4,
                     False, H * W, W, False, out_f32=True)
                tc.strict_bb_all_engine_barrier()

            # ---------------- scan-resident gate/w0 tiles
            with tc.tile_pool(name="scanbig", bufs=1) as pbig:
                gw = pbig.tile([128, 3, 24, W], bf16, tag="gw")
                gwv = gw[:]
                w0w = pbig.tile([128, 24, W], bf16, tag="w0w")
                w0v = w0w[:]
                nc.vector.memset(gwv, 0.0)
                nc.vector.memset(w0v, 0.0)

                # ---------------- gate normalization (direct into gw/w0w)
                NCH = 16
                CH = 15360 // NCH
                GP = 385 * W
                with (tc.tile_pool(name="norm3", bufs=4) as pn3,
                      tc.tile_pool(name="norm1", bufs=1) as pn1):
                    for k in range(NCH):
                        gts = []
                        for tap in range(3):
                            g = pn3.tile([128, CH], bf16, tag="gld", name="gld")
                            eng = nc.sync if tap < 2 else nc.gpsimd
                            eng.dma_start(
                                out=g[:],
                                in_=dap(Gt, tap * 8 * GP + k * CH,
                                        [(GP, 8), (24 * W, 16), (1, CH)]))
                            gts.append(g)
                        ab = []
                        for tap in range(3):
                            a = pn3.tile([128, CH], bf16, tag="gabs", name="gabs")
                            nc.scalar.activation(a[:], gts[tap][:], ACTF.Abs)
                            ab.append(a)
                        s12 = pn1.tile([128, CH], bf16, tag="s12")
                        nc.vector.tensor_tensor(out=s12[:], in0=ab[0][:], in1=ab[1][:], op=ALU.add)
                        sf = pn1.tile([128, CH], f32, tag="sf")
                        nc.vector.scalar_tensor_tensor(out=sf[:], in0=ab[2][:], scalar=1e-8,
                                                       in1=s12[:], op0=ALU.add, op1=ALU.add)
                        rs = pn1.tile([128, CH], f32, tag="rs")
                        nc.vector.reciprocal_approx_fast(out=rs[:], in_=sf[:])
                        gsl = []
                        for tap in range(3):
                            o = sap(gwv, 128, tap * 24 * W + k * CH, [(1, CH)])
                            nc.vector.tensor_tensor(out=o, in0=gts[tap][:], in1=rs[:], op=ALU.mult)
                            gsl.append(o)
                        a12 = pn1.tile([128, CH], bf16, tag="a12")
                        nc.vector.tensor_tensor(out=a12[:], in0=gsl[0], in1=gsl[1], op=ALU.add)
                        asum = pn1.tile([128, CH], bf16, tag="asum")
                        nc.vector.tensor_tensor(out=asum[:], in0=a12[:], in1=gsl[2], op=ALU.add)
                        t2 = pn1.tile([128, CH], bf16, tag="t2")
                        nc.vector.tensor_scalar(t2[:], asum[:], -1.0, 1.0, ALU.mult, ALU.add)
                        df = pn1.tile([128, CH], f32, tag="dfl")
                        nc.gpsimd.dma_start(out=df[:],
                                            in_=dap(dfeat, k * CH, [(H * W, 8), (24 * W, 16), (1, CH)]))
                        nc.vector.tensor_tensor(out=sap(w0v, 128, k * CH, [(1, CH)]),
                                                in0=t2[:], in1=df[:], op=ALU.mult)
                    # zero edge gates: tap0 (up) at row 0 of hb=0; tap2 (dn) at row 23 of hb=15
                    pstep = gwv.ap[0][0]
                    for c_ in range(8):
                        nc.sync.dma_start(
                            out=bass.AP(gwv.tensor, gwv.offset + (16 * c_) * pstep,
                                        [[pstep, 1], [1, W]]),
                            in_=sap(ztv, 1, 0, [(1, W)]))
                        nc.sync.dma_start(
                            out=bass.AP(gwv.tensor,
                                        gwv.offset + (16 * c_ + 15) * pstep + 2 * 24 * W + 23 * W,
                                        [[pstep, 1], [1, W]]),
                            in_=sap(ztv, 1, 0, [(1, W)]))
                tc.strict_bb_all_engine_barrier()

                # ---------------- SPN scan
                mask_up = [(i - 1) % 32 for i in range(32)]
                mask_dn = [(i + 1) % 32 for i in range(32)]
                TB = 32  # w0-staging chunk
                with tc.tile_pool(name="scansm", bufs=1) as psm:
                    pf = psm.tile([128, 24, WP], bf16, tag="pf")
                    pfv = pf[:]
                    nc.vector.memset(pf[:, :, 0:1], 0.0)
                    nc.vector.memset(pf[:, :, WP - 1:WP], 0.0)
                    prw = psm.tile([128, 2, TB, 24, 4], f32, tag="prw")
                    prv = prw[:]
                    state = psm.tile([128, 2, 26], f32, tag="st")
                    stv = state[:]
                    nc.vector.memset(stv, 0.0)
                    for t in range(W):
                        cur, nxt = t % 2, (t + 1) % 2
                        j = t % TB
                        bi = (t // TB) % 2
                        if j == 0:
                            # stage w0 column block into slot 3
                            nc.scalar.copy(
                                sap(prv, 128, bi * (TB * 96) + 3, [(4, 24), (96, TB)]),
                                sap(w0v, 128, t, [(W, 24), (1, TB)]))
                        base = bi * (TB * 96) + j * 96
                        taps = sap(stv, 128, cur * 26, [(1, 24), (1, 3)])
                        g_t = sap(gwv, 128, t, [(W, 24), (24 * W, 3)])
                        nc.vector.tensor_tensor(out=sap(prv, 128, base, [(4, 24), (1, 3)]),
                                                in0=g_t, in1=taps, op=ALU.mult)
                        nc.vector.tensor_reduce(out=state[:, nxt, 1:25],
                                                in_=sap(prv, 128, base, [(4, 24), (1, 4)]),
                                                axis=AX.X, op=ALU.add)
                        nc.vector.stream_shuffle(out=state[:, nxt, 0:1],
                                                 in_=state[:, nxt, 24:25], mask=mask_up)
                        nc.vector.stream_shuffle(out=state[:, nxt, 25:26],
                                                 in_=state[:, nxt, 1:2], mask=mask_dn)
                        p_t = sap(pfv, 128, 1 + t, [(WP, 24)])
                        nc.scalar.copy(p_t, state[:, nxt, 1:25])
                    # export prop -> ppad rows 1..384, full width
                    nc.sync.dma_start(
                        out=dap(ppad, WP, [(HP * WP, 8), (24 * WP, 16), (1, 24 * WP)]),
                        in_=pf[:])
                tc.strict_bb_all_engine_barrier()

            # ---------------- convc: ppad -> out (+disp, relu)
            with (tc.tile_pool(name="rhsc", bufs=3) as prhs2,
                  tc.tile_pool(name="coutc", bufs=3) as pout2,
                  tc.tile_pool(name="psumc", bufs=8, space="PSUM") as ppsum2):
                conv(prhs2, pout2, ppsum2, ppad, outp, wtl["wc"], 72, 16, None, 4, 2,
                     1, 16, 18, 24, 4,
                     False, H * W, W, False, out_f32=True, disp_add=True)

    return nc


def _prep_inputs(inputs):
    w1, b1 = _fold_bn(inputs['w1'], inputs['bn1_g'], inputs['bn1_b'], inputs['bn1_m'], inputs['bn1_v'])
    w2, b2 = _fold_bn(inputs['w2'], inputs['bn2_g'], inputs['bn2_b'], inputs['bn2_m'], inputs['bn2_v'])
    w3, b3 = _fold_bn(inputs['w3'], inputs['bn3_g'], inputs['bn3_b'], inputs['bn3_m'], inputs['bn3_v'])

    w1k = _lhsT(w1, 8, 10, 3, 1).astype(BF)                       # [30, 384]
    w2k = _lhsT(w2, 6, 8, 16, 1).astype(BF)                       # [128, 288]
    w3k = _lhsT(w3, 6, 8, 16, 1).astype(BF)
    w4k = _lhsT(inputs['w4'].astype(np.float32), 5, 7, 16, 1).astype(BF)   # [112, 360]
    wdk = _lhsT(inputs['wd'].astype(np.float32), 16, 18, 1, 1).astype(BF)  # [18, 384]
    wck = _lhsT(inputs['wc'].astype(np.float32), 16, 18, 4, 2).astype(BF)  # [72, 96]

    b1r = np.repeat(b1, 8).reshape(128, 1).astype(np.float32)
    b2r = np.repeat(b2, 6).reshape(96, 1).astype(np.float32)
    b3r = np.repeat(b3, 6).reshape(96, 1).astype(np.float32)

    maps = []
    for b in range(8):
        maps.append({
            "img": _pad_img(inputs['leftImage'][b]),
            "dpad": _pad_img(inputs['disp'][b]),
            "dispf": inputs['disp'][b, 0].astype(np.float32),
            "w1k": w1k, "w2k": w2k, "w3k": w3k, "w4k": w4k, "wdk": wdk, "wck": wck,
            "b1v": b1r, "b2v": b2r, "b3v": b3r,
        })
    return maps


def kernel(**inputs):
    from concourse.bass_utils import run_bass_kernel_spmd

    if "nc" not in _CACHE:
        _CACHE["nc"] = _build()
    nc = _CACHE["nc"]
    maps = _prep_inputs(inputs)
    res = run_bass_kernel_spmd(nc, maps, core_ids=list(range(8)))
    out = np.stack([res.results[i]["out"] for i in range(8)])[:, None].astype(np.float32)
    return out


# revision 12
# speedup vs baseline: 1.0551x; 1.0082x over previous
"""Trainium2 Bass kernel for nn_AnyNetRefinement (disparity refinement with SPN scan).

Data-parallel over batch: core b processes image b end-to-end (no collectives).
Pipeline per core:
  conv1..conv3 (3x3+BN+ReLU, bf16, row-stacked PE matmuls, DRAM-padded acts)
  conv4 -> raw gates G; convd (disp -> 8ch feature, f32)
  normalize gates (|G1|+|G2|+|G3|) writing A taps + w0 directly into
    scan-resident SBUF tiles
  SPN left-to-right scan over W=640 on VectorE (folded [128=(c,hblock), 26] state,
    3-tap multiply into a slot buffer + 4-slot tensor_reduce (slot 3 = w0,
    pre-staged by ScalarE) + stream_shuffle halos)
  convc (prop -> residual) + disp + relu -> out
"""

import numpy as np
import ml_dtypes

BF = ml_dtypes.bfloat16

H, W = 384, 640
HP, WP = 387, 642        # padded activation planes (+1 top/left, +2 bottom, +1 right)
X0S = (0, 320)
NX = 320                 # matmul free size (psum-bank safe)

_CACHE = {}


# ---------------------------------------------------------------- host helpers
def _fold_bn(wt, g, b, m, v):
    s = g / np.sqrt(v + 1e-5)
    return (wt * s.reshape(-1, 1, 1, 1)).astype(np.float32), (b - m * s).astype(np.float32)


def _lhsT(wt, r_out, r_in, cin_g, npass):
    """lhsT [K=(cin_g,yi), npass, 3, M=(cout,r_out)]."""
    cout, cin = wt.shape[0], wt.shape[1]
    K = cin_g * r_in
    M = cout * r_out
    out = np.zeros((K, npass, 3, M), np.float32)
    for p in range(npass):
        for cg in range(cin_g):
            c = p * cin_g + cg
            if c >= cin:
                continue
            for dx in range(3):
                for yi in range(r_in):
                    k = cg * r_in + yi
                    for co in range(cout):
                        for yo in range(r_out):
                            dy = yi - yo
                            if 0 <= dy <= 2:
                                out[k, p, dx, co * r_out + yo] = wt[co, c, dy, dx]
    return out.reshape(K, npass * 3 * M)


def _pad_img(x, hp=HP, wp=WP):
    out = np.zeros((x.shape[0], hp, wp), BF)
    out[:, 1:1 + H, 1:1 + W] = x.astype(BF)
    return out


# ---------------------------------------------------------------- bass builder
def _build():
    import concourse.bass as bass
    import concourse.mybir as mybir
    from concourse import tile
    from concourse.vector_clock import ScopedClock

    f32 = mybir.dt.float32
    bf16 = mybir.dt.bfloat16
    ALU = mybir.AluOpType
    ACTF = mybir.ActivationFunctionType
    AX = mybir.AxisListType

    class TC(tile.TileContext):
        # this walrus build accepts only one sync-wait per Drain; split the
        # end-of-kernel waits across several drains.
        def _drain_and_barrier(self, tick_clock, wait_clock):
            nc = self.nc
            drain_inst = nc.sync.drain()
            wait_clock.add_sem_waits(drain_inst.ins, ScopedClock({None: tick_clock.global_clock}))
            waits = list(drain_inst.ins.sync_info.on_wait)
            if len(waits) > 1:
                drain_inst.ins.sync_info.on_wait = waits[:1]
                for i in range(1, len(waits)):
                    d2 = nc.sync.drain()
                    if d2.ins.sync_info is None:
                        d2.ins.sync_info = mybir.SyncInfo(on_wait=[waits[i]], on_update=[])
                    else:
                        d2.ins.sync_info.on_wait = [waits[i]]
            nc.all_engine_barrier()
            popped = nc._tile_sem_poison_stack.pop()
            assert popped is self._sem_poison
            nc.clear_and_free_semaphores(list(self.sems.allocated().values()))
            nc.all_engine_barrier()

    def dap(t, offset, dims):
        base = t if isinstance(t, bass.AP) else t[:]
        return bass.AP(base.tensor, base.offset + offset, [list(d) for d in dims])

    def sap(tile_ap, nparts, offset, dims, pstride=1):
        pstep = tile_ap.ap[0][0]
        return bass.AP(tile_ap.tensor, tile_ap.offset + offset,
                       [[pstep * pstride, nparts]] + [list(d) for d in dims])

    nc = bass.Bass("TRN2", num_swdge_queues=4)

    img = nc.declare_dram_parameter("img", [3, HP, WP], bf16, isOutput=False)
    dpad = nc.declare_dram_parameter("dpad", [1, HP, WP], bf16, isOutput=False)
    dispf = nc.declare_dram_parameter("dispf", [H, W], f32, isOutput=False)
    w1k = nc.declare_dram_parameter("w1k", [30, 3 * 128], bf16, isOutput=False)
    w2k = nc.declare_dram_parameter("w2k", [128, 3 * 96], bf16, isOutput=False)
    w3k = nc.declare_dram_parameter("w3k", [128, 3 * 96], bf16, isOutput=False)
    w4k = nc.declare_dram_parameter("w4k", [112, 3 * 120], bf16, isOutput=False)
    wdk = nc.declare_dram_parameter("wdk", [18, 3 * 128], bf16, isOutput=False)
    wck = nc.declare_dram_parameter("wck", [72, 2 * 3 * 16], bf16, isOutput=False)
    b1v = nc.declare_dram_parameter("b1v", [128, 1], f32, isOutput=False)
    b2v = nc.declare_dram_parameter("b2v", [96, 1], f32, isOutput=False)
    b3v = nc.declare_dram_parameter("b3v", [96, 1], f32, isOutput=False)
    outp = nc.declare_dram_parameter("out", [H, W], f32, isOutput=True)

    with TC(nc) as tc:
        with (tc.tile_pool(name="dram", bufs=1, space="DRAM") as dram,
              tc.tile_pool(name="wts", bufs=1) as pw):
            act1 = dram.tile([16, HP, WP], bf16, tag="act1")
            act2 = dram.tile([16, HP, WP], bf16, tag="act2")
            act3 = dram.tile([16, HP, WP], bf16, tag="act3")
            Gt = dram.tile([24, 385, W], bf16, tag="G")
            dfeat = dram.tile([8, H, W], f32, tag="dfeat")
            ppad = dram.tile([8, HP, WP], bf16, tag="ppad")

            # ---------------- weights/biases
            wtl = {}
            for nm, prm, kk, nm3 in (("w1", w1k, 30, 3 * 128), ("w2", w2k, 128, 3 * 96),
                                     ("w3", w3k, 128, 3 * 96), ("w4", w4k, 112, 3 * 120),
                                     ("wd", wdk, 18, 3 * 128), ("wc", wck, 72, 6 * 16)):
                t = pw.tile([kk, nm3], bf16, tag=f"{nm}t", name=f"{nm}t")
                nc.sync.dma_start(out=t[:], in_=prm[:])
                wtl[nm] = t
            b1t = pw.tile([128, 1], f32, tag="b1t")
            nc.sync.dma_start(out=b1t[:], in_=b1v[:])
            b2t = pw.tile([96, 1], f32, tag="b2t")
            nc.sync.dma_start(out=b2t[:], in_=b2v[:])
            b3t = pw.tile([96, 1], f32, tag="b3t")
            nc.sync.dma_start(out=b3t[:], in_=b3v[:])

            # ---------------- zero row-borders of padded internal buffers
            zt = pw.tile([128, 2 * WP], bf16, tag="zt")
            nc.vector.memset(zt[:], 0.0)
            ztv = zt[:]
            for buf, cc in ((act1, 16), (act2, 16), (act3, 16), (ppad, 8)):
                nc.sync.dma_start(out=dap(buf, 0, [(HP * WP, cc), (1, WP)]),
                                  in_=sap(ztv, cc, 0, [(1, WP)]))
                nc.sync.dma_start(out=dap(buf, 385 * WP, [(HP * WP, cc), (1, 2 * WP)]),
                                  in_=sap(ztv, cc, 0, [(1, 2 * WP)]))

            # ---------------- generic conv
            rhs_eng = [0]

            def conv(prhs, pout, ppsum, src, dst, wt, wK, wM, btile, cin_g, npass,
                     cout, r, rin, S, GS,
                     relu, dst_plane, dst_w, dst_pad, out_f32=False, disp_add=False):
                K = cin_g * rin
                assert K == wK
                M = cout * r
                assert M == wM
                wv = wt[:]
                ow = WP if dst_pad else W
                g0 = 0
                while g0 < S:
                    nsl = min(GS, S - g0)
                    y0 = r * g0
                    rhss = []
                    for p_ in range(npass):
                        rt = prhs.tile([K, GS, WP], bf16, tag="rhs", name="rhs")
                        for sl in range(nsl):
                            eng = nc.sync if (rhs_eng[0] % 2 == 0) else nc.scalar
                            rhs_eng[0] += 1
                            eng.dma_start(
                                out=rt[:, sl, :],
                                in_=dap(src, p_ * cin_g * HP * WP + (y0 + sl * r) * WP,
                                        [(HP * WP, cin_g), (WP, rin), (1, WP)]))
                        rhss.append(rt)
                    ps = []
                    for xh in range(2):
                        for sl in range(nsl):
                            pstile = ppsum.tile([128, NX], f32, tag="ps", name="ps")
                            ps.append(pstile)
                    for xh in range(2):
                        for p_ in range(npass):
                            for dx in range(3):
                                for sl in range(nsl):
                                    nc.tensor.matmul(
                                        ps[xh * nsl + sl][:M, :],
                                        sap(wv, K, (p_ * 3 + dx) * M, [(1, M)]),
                                        rhss[p_][:, sl, X0S[xh] + dx:X0S[xh] + dx + NX],
                                        start=(p_ == 0 and dx == 0),
                                        stop=(p_ == npass - 1 and dx == 2))
                    ot = pout.tile([M, GS, ow], f32 if out_f32 else bf16, tag="cout", name="cout")
                    if dst_pad:
                        nc.vector.memset(ot[:, :, 0:1], 0.0)
                        nc.vector.memset(ot[:, :, ow - 1:ow], 0.0)
                    for xh in range(2):
                        for sl in range(nsl):
                            p = ps[xh * nsl + sl][:M, :]
                            xb = (1 if dst_pad else 0) + xh * NX
                            o = ot[:, sl, xb:xb + NX]
                            if disp_add:
                                dt_ = pout.tile([16, NX], f32, tag="dtile", name="dtile")
                                nc.gpsimd.dma_start(
                                    out=dt_[:],
                                    in_=dap(dispf, (y0 + sl * r) * W + X0S[xh], [(W, 16), (1, NX)]))
                                tmp = pout.tile([16, NX], f32, tag="ctmp", name="ctmp")
                                nc.vector.tensor_tensor(out=tmp[:], in0=p, in1=dt_[:], op=ALU.add)
                                nc.vector.tensor_scalar(o, tmp[:], 0.0, None, ALU.max)
                            elif relu:
                                nc.vector.tensor_scalar(o, p, btile[:M, :], 0.0, ALU.add, ALU.max)
                            else:
                                nc.vector.tensor_copy(o, p)
                    for sl in range(nsl):
                        nc.gpsimd.dma_start(
                            out=dap(dst, ((1 if dst_pad else 0) + y0 + sl * r) * dst_w,
                                    [(dst_plane, cout), (dst_w, r), (1, ow)]),
                            in_=ot[:, sl, :])
                    g0 += nsl

            with (tc.tile_pool(name="rhs", bufs=3) as prhs,
                  tc.tile_pool(name="cout", bufs=3) as pout,
                  tc.tile_pool(name="psum", bufs=8, space="PSUM") as ppsum):
                P3 = (prhs, pout, ppsum)
                conv(*P3, img, act1, wtl["w1"], 30, 128, b1t, 3, 1, 16, 8, 10, 48, 4,
                     True, HP * WP, WP, True)
                conv(*P3, act1, act2, wtl["w2"], 128, 96, b2t, 16, 1, 16, 6, 8, 64, 4,
                     True, HP * WP, WP, True)
                conv(*P3, act2, act3, wtl["w3"], 128, 96, b3t, 16, 1, 16, 6, 8, 64, 4,
                     True, HP * WP, WP, True)
                conv(*P3, act3, Gt, wtl["w4"], 112, 120, None, 16, 1, 24, 5, 7, 77, 4,
                     False, 385 * W, W, False)
                conv(*P3, dpad, dfeat, wtl["wd"], 18, 128, None, 1, 1, 8, 16, 18, 24, 4,
                     False, H * W, W, False, out_f32=True)
                tc.strict_bb_all_engine_barrier()

            # ---------------- scan-resident gate/w0 tiles
            with tc.tile_pool(name="scanbig", bufs=1) as pbig:
                gw = pbig.tile([128, 3, 24, W], bf16, tag="gw")
                gwv = gw[:]
                w0w = pbig.tile([128, 24, W], bf16, tag="w0w")
                w0v = w0w[:]
                nc.vector.memset(gwv, 0.0)
                nc.vector.memset(w0v, 0.0)

                # ---------------- gate normalization (direct into gw/w0w)
                NCH = 16
                CH = 15360 // NCH
                GP = 385 * W
                with (tc.tile_pool(name="norm3", bufs=4) as pn3,
                      tc.tile_pool(name="norm1", bufs=1) as pn1):
                    for k in range(NCH):
                        gts = []
                        for tap in range(3):
                            g = pn3.tile([128, CH], bf16, tag="gld", name="gld")
                            eng = (nc.sync, nc.scalar, nc.sync)[tap]
                            eng.dma_start(
                                out=g[:],
                                in_=dap(Gt, tap * 8 * GP + k * CH,
                                        [(GP, 8), (24 * W, 16), (1, CH)]))
                            gts.append(g)
                        ab = []
                        for tap in range(3):
                            a = pn3.tile([128, CH], bf16, tag="gabs", name="gabs")
                            nc.scalar.activation(a[:], gts[tap][:], ACTF.Abs)
                            ab.append(a)
                        s12 = pn1.tile([128, CH], bf16, tag="s12")
                        nc.vector.tensor_tensor(out=s12[:], in0=ab[0][:], in1=ab[1][:], op=ALU.add)
                        sf = pn1.tile([128, CH], f32, tag="sf")
                        nc.vector.scalar_tensor_tensor(out=sf[:], in0=ab[2][:], scalar=1e-8,
                                                       in1=s12[:], op0=ALU.add, op1=ALU.add)
                        rs = pn1.tile([128, CH], f32, tag="rs")
                        nc.vector.reciprocal_approx_fast(out=rs[:], in_=sf[:])
                        gsl = []
                        for tap in range(3):
                            o = sap(gwv, 128, tap * 24 * W + k * CH, [(1, CH)])
                            nc.vector.tensor_tensor(out=o, in0=gts[tap][:], in1=rs[:], op=ALU.mult)
                            gsl.append(o)
                        a12 = pn1.tile([128, CH], bf16, tag="a12")
                        nc.vector.tensor_tensor(out=a12[:], in0=gsl[0], in1=gsl[1], op=ALU.add)
                        asum = pn1.tile([128, CH], bf16, tag="asum")
                        nc.vector.tensor_tensor(out=asum[:], in0=a12[:], in1=gsl[2], op=ALU.add)
                        t2 = pn1.tile([128, CH], bf16, tag="t2")
                        nc.vector.tensor_scalar(t2[:], asum[:], -1.0, 1.0, ALU.mult, ALU.add)
                        df = pn1.tile([128, CH], f32, tag="dfl")
                        nc.gpsimd.dma_start(out=df[:],
                                            in_=dap(dfeat, k * CH, [(H * W, 8), (24 * W, 16), (1, CH)]))
                        nc.vector.tensor_tensor(out=sap(w0v, 128, k * CH, [(1, CH)]),
                                                in0=t2[:], in1=df[:], op=ALU.mult)
                    # zero edge gates: tap0 (up) at row 0 of hb=0; tap2 (dn) at row 23 of hb=15
                    pstep = gwv.ap[0][0]
                    for c_ in range(8):
                        nc.sync.dma_start(
                            out=bass.AP(gwv.tensor, gwv.offset + (16 * c_) * pstep,
                                        [[pstep, 1], [1, W]]),
                            in_=sap(ztv, 1, 0, [(1, W)]))
                        nc.sync.dma_start(
                            out=bass.AP(gwv.tensor,
                                        gwv.offset + (16 * c_ + 15) * pstep + 2 * 24 * W + 23 * W,
                                        [[pstep, 1], [1, W]]),
                            in_=sap(ztv, 1, 0, [(1, W)]))
                tc.strict_bb_all_engine_barrier()

                # ---------------- SPN scan
                mask_up = [(i - 1) % 32 for i in range(32)]
                mask_dn = [(i + 1) % 32 for i in range(32)]
                TB = 32  # w0-staging chunk
                with tc.tile_pool(name="scansm", bufs=1) as psm:
                    pf = psm.tile([128, 24, WP], bf16, tag="pf")
                    pfv = pf[:]
                    nc.vector.memset(pf[:, :, 0:1], 0.0)
                    nc.vector.memset(pf[:, :, WP - 1:WP], 0.0)
                    prw = psm.tile([128, 2, TB, 24, 4], f32, tag="prw")
                    prv = prw[:]
                    state = psm.tile([128, 2, 26], f32, tag="st")
                    stv = state[:]
                    nc.vector.memset(stv, 0.0)
                    for t in range(W):
                        cur, nxt = t % 2, (t + 1) % 2
                        j = t % TB
                        bi = (t // TB) % 2
                        if j == 0:
                            # stage w0 column block into slot 3
                            nc.scalar.copy(
                                sap(prv, 128, bi * (TB * 96) + 3, [(4, 24), (96, TB)]),
                                sap(w0v, 128, t, [(W, 24), (1, TB)]))
                        base = bi * (TB * 96) + j * 96
                        taps = sap(stv, 128, cur * 26, [(1, 24), (1, 3)])
                        g_t = sap(gwv, 128, t, [(W, 24), (24 * W, 3)])
                        nc.vector.tensor_tensor(out=sap(prv, 128, base, [(4, 24), (1, 3)]),
                                                in0=g_t, in1=taps, op=ALU.mult)
                        nc.vector.tensor_reduce(out=state[:, nxt, 1:25],
                                                in_=sap(prv, 128, base, [(4, 24), (1, 4)]),
                                                axis=AX.X, op=ALU.add)
                        nc.vector.stream_shuffle(out=state[:, nxt, 0:1],
                                                 in_=state[:, nxt, 24:25], mask=mask_up)
                        nc.vector.stream_shuffle(out=state[:, nxt, 25:26],
                                                 in_=state[:, nxt, 1:2], mask=mask_dn)
                        p_t = sap(pfv, 128, 1 + t, [(WP, 24)])
                        nc.scalar.copy(p_t, state[:, nxt, 1:25])
                    # export prop -> ppad rows 1..384, full width
                    nc.sync.dma_start(
                        out=dap(ppad, WP, [(HP * WP, 8), (24 * WP, 16), (1, 24 * WP)]),
                        in_=pf[:])
                tc.strict_bb_all_engine_barrier()

            # ---------------- convc: ppad -> out (+disp, relu)
            with (tc.tile_pool(name="rhsc", bufs=3) as prhs2,
                  tc.tile_pool(name="coutc", bufs=3) as pout2,
                  tc.tile_pool(name="psumc", bufs=8, space="PSUM") as ppsum2):
                conv(prhs2, pout2, ppsum2, ppad, outp, wtl["wc"], 72, 16, None, 4, 2,
                     1, 16, 18, 24, 4,
                     False, H * W, W, False, out_f32=True, disp_add=True)

    return nc


def _prep_inputs(inputs):
    w1, b1 = _fold_bn(inputs['w1'], inputs['bn1_g'], inputs['bn1_b'], inputs['bn1_m'], inputs['bn1_v'])
    w2, b2 = _fold_bn(inputs['w2'], inputs['bn2_g'], inputs['bn2_b'], inputs['bn2_m'], inputs['bn2_v'])
    w3, b3 = _fold_bn(inputs['w3'], inputs['bn3_g'], inputs['bn3_b'], inputs['bn3_m'], inputs['bn3_v'])

    w1k = _lhsT(w1, 8, 10, 3, 1).astype(BF)                       # [30, 384]
    w2k = _lhsT(w2, 6, 8, 16, 1).astype(BF)                       # [128, 288]
    w3k = _lhsT(w3, 6, 8, 16, 1).astype(BF)
    w4k = _lhsT(inputs['w4'].astype(np.float32), 5, 7, 16, 1).astype(BF)   # [112, 360]
    wdk = _lhsT(inputs['wd'].astype(np.float32), 16, 18, 1, 1).astype(BF)  # [18, 384]
    wck = _lhsT(inputs['wc'].astype(np.float32), 16, 18, 4, 2).astype(BF)  # [72, 96]

    b1r = np.repeat(b1, 8).reshape(128, 1).astype(np.float32)
    b2r = np.repeat(b2, 6).reshape(96, 1).astype(np.float32)
    b3r = np.repeat(b3, 6).reshape(96, 1).astype(np.float32)

    maps = []
    for b in range(8):
        maps.append({
            "img": _pad_img(inputs['leftImage'][b]),
            "dpad": _pad_img(inputs['disp'][b]),
            "dispf": inputs['disp'][b, 0].astype(np.float32),
            "w1k": w1k, "w2k": w2k, "w3k": w3k, "w4k": w4k, "wdk": wdk, "wck": wck,
            "b1v": b1r, "b2v": b2r, "b3v": b3r,
        })
    return maps


def kernel(**inputs):
    from concourse.bass_utils import run_bass_kernel_spmd

    if "nc" not in _CACHE:
        _CACHE["nc"] = _build()
    nc = _CACHE["nc"]
    maps = _prep_inputs(inputs)
    res = run_bass_kernel_spmd(nc, maps, core_ids=list(range(8)))
    out = np.stack([res.results[i]["out"] for i in range(8)])[:, None].astype(np.float32)
    return out


# revision 13
# speedup vs baseline: 1.0557x; 1.0006x over previous
"""Trainium2 Bass kernel for nn_AnyNetRefinement (disparity refinement with SPN scan).

Data-parallel over batch: core b processes image b end-to-end (no collectives).
Pipeline per core:
  conv1..conv3 (3x3+BN+ReLU, bf16, row-stacked PE matmuls, DRAM-padded acts)
  conv4 -> raw gates G; convd (disp -> 8ch feature, f32)
  normalize gates (|G1|+|G2|+|G3|) writing A taps + w0 directly into
    scan-resident SBUF tiles
  SPN left-to-right scan over W=640 on VectorE (folded [128=(c,hblock), 26] state,
    3-tap multiply into a slot buffer + 4-slot tensor_reduce (slot 3 = w0,
    pre-staged by ScalarE) + stream_shuffle halos)
  convc (prop -> residual) + disp + relu -> out
"""

import numpy as np
import ml_dtypes

BF = ml_dtypes.bfloat16

H, W = 384, 640
HP, WP = 387, 642        # padded activation planes (+1 top/left, +2 bottom, +1 right)
X0S = (0, 320)
NX = 320                 # matmul free size (psum-bank safe)

_CACHE = {}


# ---------------------------------------------------------------- host helpers
def _fold_bn(wt, g, b, m, v):
    s = g / np.sqrt(v + 1e-5)
    return (wt * s.reshape(-1, 1, 1, 1)).astype(np.float32), (b - m * s).astype(np.float32)


def _lhsT(wt, r_out, r_in, cin_g, npass):
    """lhsT [K=(cin_g,yi), npass, 3, M=(cout,r_out)]."""
    cout, cin = wt.shape[0], wt.shape[1]
    K = cin_g * r_in
    M = cout * r_out
    out = np.zeros((K, npass, 3, M), np.float32)
    for p in range(npass):
        for cg in range(cin_g):
            c = p * cin_g + cg
            if c >= cin:
                continue
            for dx in range(3):
                for yi in range(r_in):
                    k = cg * r_in + yi
                    for co in range(cout):
                        for yo in range(r_out):
                            dy = yi - yo
                            if 0 <= dy <= 2:
                                out[k, p, dx, co * r_out + yo] = wt[co, c, dy, dx]
    return out.reshape(K, npass * 3 * M)


def _pad_img(x, hp=HP, wp=WP):
    out = np.zeros((x.shape[0], hp, wp), BF)
    out[:, 1:1 + H, 1:1 + W] = x.astype(BF)
    return out


# ---------------------------------------------------------------- bass builder
def _build():
    import concourse.bass as bass
    import concourse.mybir as mybir
    from concourse import tile
    from concourse.vector_clock import ScopedClock

    f32 = mybir.dt.float32
    bf16 = mybir.dt.bfloat16
    ALU = mybir.AluOpType
    ACTF = mybir.ActivationFunctionType
    AX = mybir.AxisListType

    class TC(tile.TileContext):
        # this walrus build accepts only one sync-wait per Drain; split the
        # end-of-kernel waits across several drains.
        def _drain_and_barrier(self, tick_clock, wait_clock):
            nc = self.nc
            drain_inst = nc.sync.drain()
            wait_clock.add_sem_waits(drain_inst.ins, ScopedClock({None: tick_clock.global_clock}))
            waits = list(drain_inst.ins.sync_info.on_wait)
            if len(waits) > 1:
                drain_inst.ins.sync_info.on_wait = waits[:1]
                for i in range(1, len(waits)):
                    d2 = nc.sync.drain()
                    if d2.ins.sync_info is None:
                        d2.ins.sync_info = mybir.SyncInfo(on_wait=[waits[i]], on_update=[])
                    else:
                        d2.ins.sync_info.on_wait = [waits[i]]
            nc.all_engine_barrier()
            popped = nc._tile_sem_poison_stack.pop()
            assert popped is self._sem_poison
            nc.clear_and_free_semaphores(list(self.sems.allocated().values()))
            nc.all_engine_barrier()

    def dap(t, offset, dims):
        base = t if isinstance(t, bass.AP) else t[:]
        return bass.AP(base.tensor, base.offset + offset, [list(d) for d in dims])

    def sap(tile_ap, nparts, offset, dims, pstride=1):
        pstep = tile_ap.ap[0][0]
        return bass.AP(tile_ap.tensor, tile_ap.offset + offset,
                       [[pstep * pstride, nparts]] + [list(d) for d in dims])

    nc = bass.Bass("TRN2", num_swdge_queues=4)

    img = nc.declare_dram_parameter("img", [3, HP, WP], bf16, isOutput=False)
    dpad = nc.declare_dram_parameter("dpad", [1, HP, WP], bf16, isOutput=False)
    dispf = nc.declare_dram_parameter("dispf", [H, W], f32, isOutput=False)
    w1k = nc.declare_dram_parameter("w1k", [30, 3 * 128], bf16, isOutput=False)
    w2k = nc.declare_dram_parameter("w2k", [128, 3 * 96], bf16, isOutput=False)
    w3k = nc.declare_dram_parameter("w3k", [128, 3 * 96], bf16, isOutput=False)
    w4k = nc.declare_dram_parameter("w4k", [112, 3 * 120], bf16, isOutput=False)
    wdk = nc.declare_dram_parameter("wdk", [18, 3 * 128], bf16, isOutput=False)
    wck = nc.declare_dram_parameter("wck", [72, 2 * 3 * 16], bf16, isOutput=False)
    b1v = nc.declare_dram_parameter("b1v", [128, 1], f32, isOutput=False)
    b2v = nc.declare_dram_parameter("b2v", [96, 1], f32, isOutput=False)
    b3v = nc.declare_dram_parameter("b3v", [96, 1], f32, isOutput=False)
    outp = nc.declare_dram_parameter("out", [H, W], f32, isOutput=True)

    with TC(nc) as tc:
        with (tc.tile_pool(name="dram", bufs=1, space="DRAM") as dram,
              tc.tile_pool(name="wts", bufs=1) as pw):
            act1 = dram.tile([16, HP, WP], bf16, tag="act1")
            act2 = dram.tile([16, HP, WP], bf16, tag="act2")
            act3 = dram.tile([16, HP, WP], bf16, tag="act3")
            Gt = dram.tile([24, 385, W], bf16, tag="G")
            dfeat = dram.tile([8, H, W], f32, tag="dfeat")
            ppad = dram.tile([8, HP, WP], bf16, tag="ppad")

            # ---------------- weights/biases
            wtl = {}
            for nm, prm, kk, nm3 in (("w1", w1k, 30, 3 * 128), ("w2", w2k, 128, 3 * 96),
                                     ("w3", w3k, 128, 3 * 96), ("w4", w4k, 112, 3 * 120),
                                     ("wd", wdk, 18, 3 * 128), ("wc", wck, 72, 6 * 16)):
                t = pw.tile([kk, nm3], bf16, tag=f"{nm}t", name=f"{nm}t")
                nc.sync.dma_start(out=t[:], in_=prm[:])
                wtl[nm] = t
            b1t = pw.tile([128, 1], f32, tag="b1t")
            nc.sync.dma_start(out=b1t[:], in_=b1v[:])
            b2t = pw.tile([96, 1], f32, tag="b2t")
            nc.sync.dma_start(out=b2t[:], in_=b2v[:])
            b3t = pw.tile([96, 1], f32, tag="b3t")
            nc.sync.dma_start(out=b3t[:], in_=b3v[:])

            # ---------------- zero row-borders of padded internal buffers
            zt = pw.tile([128, 2 * WP], bf16, tag="zt")
            nc.vector.memset(zt[:], 0.0)
            ztv = zt[:]
            for buf, cc in ((act1, 16), (act2, 16), (act3, 16), (ppad, 8)):
                nc.sync.dma_start(out=dap(buf, 0, [(HP * WP, cc), (1, WP)]),
                                  in_=sap(ztv, cc, 0, [(1, WP)]))
                nc.sync.dma_start(out=dap(buf, 385 * WP, [(HP * WP, cc), (1, 2 * WP)]),
                                  in_=sap(ztv, cc, 0, [(1, 2 * WP)]))

            # ---------------- generic conv
            rhs_eng = [0]

            def conv(prhs, pout, ppsum, src, dst, wt, wK, wM, btile, cin_g, npass,
                     cout, r, rin, S, GS,
                     relu, dst_plane, dst_w, dst_pad, out_f32=False, disp_add=False):
                K = cin_g * rin
                assert K == wK
                M = cout * r
                assert M == wM
                wv = wt[:]
                ow = WP if dst_pad else W
                g0 = 0
                while g0 < S:
                    nsl = min(GS, S - g0)
                    y0 = r * g0
                    rhss = []
                    for p_ in range(npass):
                        rt = prhs.tile([K, GS, WP], bf16, tag="rhs", name="rhs")
                        for sl in range(nsl):
                            eng = nc.sync
                            rhs_eng[0] += 1
                            eng.dma_start(
                                out=rt[:, sl, :],
                                in_=dap(src, p_ * cin_g * HP * WP + (y0 + sl * r) * WP,
                                        [(HP * WP, cin_g), (WP, rin), (1, WP)]))
                        rhss.append(rt)
                    ps = []
                    for xh in range(2):
                        for sl in range(nsl):
                            pstile = ppsum.tile([128, NX], f32, tag="ps", name="ps")
                            ps.append(pstile)
                    for xh in range(2):
                        for p_ in range(npass):
                            for dx in range(3):
                                for sl in range(nsl):
                                    nc.tensor.matmul(
                                        ps[xh * nsl + sl][:M, :],
                                        sap(wv, K, (p_ * 3 + dx) * M, [(1, M)]),
                                        rhss[p_][:, sl, X0S[xh] + dx:X0S[xh] + dx + NX],
                                        start=(p_ == 0 and dx == 0),
                                        stop=(p_ == npass - 1 and dx == 2))
                    ot = pout.tile([M, GS, ow], f32 if out_f32 else bf16, tag="cout", name="cout")
                    if dst_pad:
                        nc.vector.memset(ot[:, :, 0:1], 0.0)
                        nc.vector.memset(ot[:, :, ow - 1:ow], 0.0)
                    for xh in range(2):
                        for sl in range(nsl):
                            p = ps[xh * nsl + sl][:M, :]
                            xb = (1 if dst_pad else 0) + xh * NX
                            o = ot[:, sl, xb:xb + NX]
                            if disp_add:
                                dt_ = pout.tile([16, NX], f32, tag="dtile", name="dtile")
                                nc.sync.dma_start(
                                    out=dt_[:],
                                    in_=dap(dispf, (y0 + sl * r) * W + X0S[xh], [(W, 16), (1, NX)]))
                                tmp = pout.tile([16, NX], f32, tag="ctmp", name="ctmp")
                                nc.vector.tensor_tensor(out=tmp[:], in0=p, in1=dt_[:], op=ALU.add)
                                nc.vector.tensor_scalar(o, tmp[:], 0.0, None, ALU.max)
                            elif relu:
                                nc.vector.tensor_scalar(o, p, btile[:M, :], 0.0, ALU.add, ALU.max)
                            else:
                                nc.vector.tensor_copy(o, p)
                    for sl in range(nsl):
                        nc.scalar.dma_start(
                            out=dap(dst, ((1 if dst_pad else 0) + y0 + sl * r) * dst_w,
                                    [(dst_plane, cout), (dst_w, r), (1, ow)]),
                            in_=ot[:, sl, :])
                    g0 += nsl

            with (tc.tile_pool(name="rhs", bufs=4) as prhs,
                  tc.tile_pool(name="cout", bufs=4) as pout,
                  tc.tile_pool(name="psum", bufs=8, space="PSUM") as ppsum):
                P3 = (prhs, pout, ppsum)
                conv(*P3, img, act1, wtl["w1"], 30, 128, b1t, 3, 1, 16, 8, 10, 48, 2,
                     True, HP * WP, WP, True)
                conv(*P3, act1, act2, wtl["w2"], 128, 96, b2t, 16, 1, 16, 6, 8, 64, 2,
                     True, HP * WP, WP, True)
                conv(*P3, act2, act3, wtl["w3"], 128, 96, b3t, 16, 1, 16, 6, 8, 64, 2,
                     True, HP * WP, WP, True)
                conv(*P3, act3, Gt, wtl["w4"], 112, 120, None, 16, 1, 24, 5, 7, 77, 2,
                     False, 385 * W, W, False)
                conv(*P3, dpad, dfeat, wtl["wd"], 18, 128, None, 1, 1, 8, 16, 18, 24, 2,
                     False, H * W, W, False, out_f32=True)
                tc.strict_bb_all_engine_barrier()

            # ---------------- scan-resident gate/w0 tiles
            with tc.tile_pool(name="scanbig", bufs=1) as pbig:
                gw = pbig.tile([128, 3, 24, W], bf16, tag="gw")
                gwv = gw[:]
                w0w = pbig.tile([128, 24, W], bf16, tag="w0w")
                w0v = w0w[:]
                nc.vector.memset(gwv, 0.0)
                nc.vector.memset(w0v, 0.0)

                # ---------------- gate normalization (direct into gw/w0w)
                NCH = 16
                CH = 15360 // NCH
                GP = 385 * W
                with (tc.tile_pool(name="norm3", bufs=4) as pn3,
                      tc.tile_pool(name="norm1", bufs=1) as pn1):
                    for k in range(NCH):
                        gts = []
                        for tap in range(3):
                            g = pn3.tile([128, CH], bf16, tag="gld", name="gld")
                            eng = (nc.sync, nc.scalar, nc.sync)[tap]
                            eng.dma_start(
                                out=g[:],
                                in_=dap(Gt, tap * 8 * GP + k * CH,
                                        [(GP, 8), (24 * W, 16), (1, CH)]))
                            gts.append(g)
                        ab = []
                        for tap in range(3):
                            a = pn3.tile([128, CH], bf16, tag="gabs", name="gabs")
                            nc.scalar.activation(a[:], gts[tap][:], ACTF.Abs)
                            ab.append(a)
                        s12 = pn1.tile([128, CH], bf16, tag="s12")
                        nc.vector.tensor_tensor(out=s12[:], in0=ab[0][:], in1=ab[1][:], op=ALU.add)
                        sf = pn1.tile([128, CH], f32, tag="sf")
                        nc.vector.scalar_tensor_tensor(out=sf[:], in0=ab[2][:], scalar=1e-8,
                                                       in1=s12[:], op0=ALU.add, op1=ALU.add)
                        rs = pn1.tile([128, CH], f32, tag="rs")
                        nc.vector.reciprocal_approx_fast(out=rs[:], in_=sf[:])
                        gsl = []
                        for tap in range(3):
                            o = sap(gwv, 128, tap * 24 * W + k * CH, [(1, CH)])
                            nc.vector.tensor_tensor(out=o, in0=gts[tap][:], in1=rs[:], op=ALU.mult)
                            gsl.append(o)
                        a12 = pn1.tile([128, CH], bf16, tag="a12")
                        nc.vector.tensor_tensor(out=a12[:], in0=gsl[0], in1=gsl[1], op=ALU.add)
                        asum = pn1.tile([128, CH], bf16, tag="asum")
                        nc.vector.tensor_tensor(out=asum[:], in0=a12[:], in1=gsl[2], op=ALU.add)
                        t2 = pn1.tile([128, CH], bf16, tag="t2")
                        nc.vector.tensor_scalar(t2[:], asum[:], -1.0, 1.0, ALU.mult, ALU.add)
                        df = pn1.tile([128, CH], f32, tag="dfl")
                        nc.sync.dma_start(out=df[:],
                                            in_=dap(dfeat, k * CH, [(H * W, 8), (24 * W, 16), (1, CH)]))
                        nc.vector.tensor_tensor(out=sap(w0v, 128, k * CH, [(1, CH)]),
                                                in0=t2[:], in1=df[:], op=ALU.mult)
                    # zero edge gates: tap0 (up) at row 0 of hb=0; tap2 (dn) at row 23 of hb=15
                    pstep = gwv.ap[0][0]
                    for c_ in range(8):
                        nc.sync.dma_start(
                            out=bass.AP(gwv.tensor, gwv.offset + (16 * c_) * pstep,
                                        [[pstep, 1], [1, W]]),
                            in_=sap(ztv, 1, 0, [(1, W)]))
                        nc.sync.dma_start(
                            out=bass.AP(gwv.tensor,
                                        gwv.offset + (16 * c_ + 15) * pstep + 2 * 24 * W + 23 * W,
                                        [[pstep, 1], [1, W]]),
                            in_=sap(ztv, 1, 0, [(1, W)]))
                tc.strict_bb_all_engine_barrier()

                # ---------------- SPN scan
                mask_up = [(i - 1) % 32 for i in range(32)]
                mask_dn = [(i + 1) % 32 for i in range(32)]
                TB = 32  # w0-staging chunk
                with tc.tile_pool(name="scansm", bufs=1) as psm:
                    pf = psm.tile([128, 24, WP], bf16, tag="pf")
                    pfv = pf[:]
                    nc.vector.memset(pf[:, :, 0:1], 0.0)
                    nc.vector.memset(pf[:, :, WP - 1:WP], 0.0)
                    prw = psm.tile([128, 2, TB, 24, 4], f32, tag="prw")
                    prv = prw[:]
                    state = psm.tile([128, 2, 26], f32, tag="st")
                    stv = state[:]
                    nc.vector.memset(stv, 0.0)
                    for t in range(W):
                        cur, nxt = t % 2, (t + 1) % 2
                        j = t % TB
                        bi = (t // TB) % 2
                        if j == 0:
                            # stage w0 column block into slot 3
                            nc.scalar.copy(
                                sap(prv, 128, bi * (TB * 96) + 3, [(4, 24), (96, TB)]),
                                sap(w0v, 128, t, [(W, 24), (1, TB)]))
                        base = bi * (TB * 96) + j * 96
                        taps = sap(stv, 128, cur * 26, [(1, 24), (1, 3)])
                        g_t = sap(gwv, 128, t, [(W, 24), (24 * W, 3)])
                        nc.vector.tensor_tensor(out=sap(prv, 128, base, [(4, 24), (1, 3)]),
                                                in0=g_t, in1=taps, op=ALU.mult)
                        nc.vector.tensor_reduce(out=state[:, nxt, 1:25],
                                                in_=sap(prv, 128, base, [(4, 24), (1, 4)]),
                                                axis=AX.X, op=ALU.add)
                        nc.vector.stream_shuffle(out=state[:, nxt, 0:1],
                                                 in_=state[:, nxt, 24:25], mask=mask_up)
                        nc.vector.stream_shuffle(out=state[:, nxt, 25:26],
                                                 in_=state[:, nxt, 1:2], mask=mask_dn)
                        p_t = sap(pfv, 128, 1 + t, [(WP, 24)])
                        nc.scalar.copy(p_t, state[:, nxt, 1:25])
                    # export prop -> ppad rows 1..384, full width
                    nc.sync.dma_start(
                        out=dap(ppad, WP, [(HP * WP, 8), (24 * WP, 16), (1, 24 * WP)]),
                        in_=pf[:])
                tc.strict_bb_all_engine_barrier()

            # ---------------- convc: ppad -> out (+disp, relu)
            with (tc.tile_pool(name="rhsc", bufs=4) as prhs2,
                  tc.tile_pool(name="coutc", bufs=4) as pout2,
                  tc.tile_pool(name="psumc", bufs=8, space="PSUM") as ppsum2):
                conv(prhs2, pout2, ppsum2, ppad, outp, wtl["wc"], 72, 16, None, 4, 2,
                     1, 16, 18, 24, 2,
                     False, H * W, W, False, out_f32=True, disp_add=True)

    return nc


def _prep_inputs(inputs):
    w1, b1 = _fold_bn(inputs['w1'], inputs['bn1_g'], inputs['bn1_b'], inputs['bn1_m'], inputs['bn1_v'])
    w2, b2 = _fold_bn(inputs['w2'], inputs['bn2_g'], inputs['bn2_b'], inputs['bn2_m'], inputs['bn2_v'])
    w3, b3 = _fold_bn(inputs['w3'], inputs['bn3_g'], inputs['bn3_b'], inputs['bn3_m'], inputs['bn3_v'])

    w1k = _lhsT(w1, 8, 10, 3, 1).astype(BF)                       # [30, 384]
    w2k = _lhsT(w2, 6, 8, 16, 1).astype(BF)                       # [128, 288]
    w3k = _lhsT(w3, 6, 8, 16, 1).astype(BF)
    w4k = _lhsT(inputs['w4'].astype(np.float32), 5, 7, 16, 1).astype(BF)   # [112, 360]
    wdk = _lhsT(inputs['wd'].astype(np.float32), 16, 18, 1, 1).astype(BF)  # [18, 384]
    wck = _lhsT(inputs['wc'].astype(np.float32), 16, 18, 4, 2).astype(BF)  # [72, 96]

    b1r = np.repeat(b1, 8).reshape(128, 1).astype(np.float32)
    b2r = np.repeat(b2, 6).reshape(96, 1).astype(np.float32)
    b3r = np.repeat(b3, 6).reshape(96, 1).astype(np.float32)

    maps = []
    for b in range(8):
        maps.append({
            "img": _pad_img(inputs['leftImage'][b]),
            "dpad": _pad_img(inputs['disp'][b]),
            "dispf": inputs['disp'][b, 0].astype(np.float32),
            "w1k": w1k, "w2k": w2k, "w3k": w3k, "w4k": w4k, "wdk": wdk, "wck": wck,
            "b1v": b1r, "b2v": b2r, "b3v": b3r,
        })
    return maps


def kernel(**inputs):
    from concourse.bass_utils import run_bass_kernel_spmd

    if "nc" not in _CACHE:
        _CACHE["nc"] = _build()
    nc = _CACHE["nc"]
    maps = _prep_inputs(inputs)
    res = run_bass_kernel_spmd(nc, maps, core_ids=list(range(8)))
    out = np.stack([res.results[i]["out"] for i in range(8)])[:, None].astype(np.float32)
    return out
